# revision 1
# baseline (speedup 1.0000x reference)
"""Trainium2 Bass kernel for nn_ForceMatchingLoss (batch-data-parallel over 8 NeuronCores).

Full inputs (B=256) are sharded along the batch dimension: core i computes
batches [32*i, 32*i+32) and returns [sum_b fd_b, sum_b cons_b]; the host
sums the 8 partials and divides by 256 (the loss is a batch mean, so the
"all-reduce" is a trivial host-side sum of 8 scalars).

v2: bf16 matmul pipeline (cast k/v once on-chip, interleaved [k|v] layout so
DMA moves 2KB-contiguous lines), bf16 PE transposes, jac streams trimmed to
the needed halves, cg chain batched per group (one cross-scores matmul with
a block-diagonal mask instead of dozens of tiny fp32 matmuls), and
elementwise work spread across Act/DVE/Pool.
"""

import numpy as np


# ---------------------------------------------------------------------------
# Workaround for this walrus build: CTRL-type instructions (Drain) only accept
# a single sync-wait; TileContext's tail drain aggregates one wait per logical
# processor.  Split the waits across a chain of drains.
# ---------------------------------------------------------------------------
def _install_drain_fix():
    import concourse.tile as tile
    from bass_rust import ScopedClock, SyncInfo

    if getattr(tile.TileContext, "_drain_fix_installed", False):
        return

    def _drain_and_barrier(self, tick_clock, wait_clock):
        drain_inst = self.nc.sync.drain()
        wait_clock.add_sem_waits(
            drain_inst.ins, ScopedClock({None: tick_clock.global_clock})
        )
        si = drain_inst.ins.sync_info
        waits = list(si.on_wait) if si is not None else []
        if len(waits) > 1:
            drain_inst.ins.sync_info = SyncInfo(
                on_wait=waits[:1], on_update=list(si.on_update)
            )
            for i in range(1, len(waits)):
                d = self.nc.sync.drain()
                d.ins.sync_info = SyncInfo(on_wait=waits[i : i + 1], on_update=[])

        self.nc.all_engine_barrier()
        popped = self.nc._tile_sem_poison_stack.pop()
        assert popped is self._sem_poison
        self.nc.clear_and_free_semaphores(list(self.sems.allocated().values()))
        self.nc.all_engine_barrier()

    tile.TileContext._drain_and_barrier = _drain_and_barrier
    tile.TileContext._drain_fix_installed = True


import concourse.bass as bass
import concourse.tile as tile
from concourse import mybir
from concourse.bass import ds, ts
from concourse.masks import make_identity

FP32 = mybir.dt.float32
BF16 = mybir.dt.bfloat16
F32R = mybir.dt.float32r
F8 = mybir.dt.float8e4
DR = mybir.MatmulPerfMode.DoubleRow
AX = mybir.AxisListType
ALU = mybir.AluOpType
ACTF = mybir.ActivationFunctionType

B = 32          # batches per core
Q = 16
S = 512
M = 8
D = 128
NCH = 4         # s chunks of 128
GB = 4          # batches per group (32-row bands in the scores psum tile)
NG = B // GB    # 8 groups
SCALE = float(D) ** -0.5
EPS = 1e-8
QD = float(Q * D)


def r(ap):
    return ap.bitcast(F32R)


def host_consts():
    """Constant mask tensors supplied as extra kernel inputs."""
    # cmask [128, 32]: block-diagonal valid mask for the batched cg scores.
    # row = 32*j + qi (qi valid when < 16), col = 8*j' + m; 1 iff j==j', qi<16.
    cm = np.zeros((128, GB * M), dtype=np.float32)
    for j in range(GB):
        cm[32 * j : 32 * j + Q, M * j : M * j + M] = 1.0
    # qmask1 / qmaskS [128,1]: per-partition valid-row masks for the okb/okcg
    # psum->sbuf copies (junk rows zeroed; kbar half also folds -SCALE).
    q1 = np.zeros((128, 1), dtype=np.float32)
    qs = np.zeros((128, 1), dtype=np.float32)
    for j in range(GB):
        q1[32 * j : 32 * j + Q] = 1.0
        qs[32 * j : 32 * j + Q] = -SCALE
    return {"cmask": cm, "qmask1": q1, "qmaskS": qs}


def build_nc(debug_dump=False):
    nc = bass.Bass("TRN2", target_bir_lowering=False, debug=False)
    q_d = nc.dram_tensor("queries", [B, Q, D], FP32, kind="ExternalInput").ap()
    k_d = nc.dram_tensor("keys", [B, S, D], FP32, kind="ExternalInput").ap()
    v_d = nc.dram_tensor("values", [B, S, D], FP32, kind="ExternalInput").ap()
    kcg_d = nc.dram_tensor("k_cg", [B, M, D], FP32, kind="ExternalInput").ap()
    vcg_d = nc.dram_tensor("v_cg", [B, M, D], FP32, kind="ExternalInput").ap()
    cm_d = nc.dram_tensor("cmask", [128, GB * M], FP32, kind="ExternalInput").ap()
    q1_d = nc.dram_tensor("qmask1", [128, 1], FP32, kind="ExternalInput").ap()
    qs_d = nc.dram_tensor("qmaskS", [128, 1], FP32, kind="ExternalInput").ap()
    out_d = nc.dram_tensor("out", [1, 2], FP32, kind="ExternalOutput").ap()
    if debug_dump:
        dbg_pt = nc.dram_tensor("dbg_pt", [128, 512], F8, kind="ExternalOutput").ap()
        dbg_okb = nc.dram_tensor("dbg_okb", [128, 256], BF16, kind="ExternalOutput").ap()
        dbg_okcg = nc.dram_tensor("dbg_okcg", [128, 256], BF16, kind="ExternalOutput").ap()
        dbg_jp = nc.dram_tensor("dbg_jp", [128, 256], FP32, kind="ExternalOutput").ap()
        dbg_acc = nc.dram_tensor("dbg_acc", [128, 128], FP32, kind="ExternalOutput").ap()
        dbg_pcg = nc.dram_tensor("dbg_pcg", [32, 128], FP32, kind="ExternalOutput").ap()
        dbg_okraw = nc.dram_tensor("dbg_okraw", [128, 256], FP32, kind="ExternalOutput").ap()
        dbg_jc_pre = nc.dram_tensor("dbg_jc_pre", [128, 128], FP32, kind="ExternalOutput").ap()
        dbg_jc_post = nc.dram_tensor("dbg_jc_post", [128, 128], FP32, kind="ExternalOutput").ap()
        dbg_cgs = nc.dram_tensor("dbg_cgs", [128, 32], FP32, kind="ExternalOutput").ap()
        dbg_em = nc.dram_tensor("dbg_em", [128, 32], FP32, kind="ExternalOutput").ap()

    with tile.TileContext(nc) as tc:
        with (
            tc.tile_pool(name="const", bufs=1) as constp,
            tc.tile_pool(name="kraw", bufs=3) as krawp,
            tc.tile_pool(name="kvb", bufs=3) as kvbp,
            tc.tile_pool(name="kt", bufs=5) as ktp,
            tc.tile_pool(name="sm", bufs=3) as smp,
            tc.tile_pool(name="small", bufs=4) as smallp,
            tc.tile_pool(name="ok", bufs=3) as okp,
            tc.tile_pool(name="scr", bufs=6) as scrp,
            tc.tile_pool(name="psSC", bufs=2, space="PSUM") as psSC,
            tc.tile_pool(name="psT", bufs=2, space="PSUM") as psT,
            tc.tile_pool(name="psOK", bufs=2, space="PSUM") as psOK,
            tc.tile_pool(name="psJ", bufs=2, space="PSUM") as psJ,
        ):
            # prefetch the first two groups' k/v so compute starts early
            pre_kr, pre_vr = [], []
            for g in range(2):
                kr0 = krawp.tile([128, GB, NCH, 128], FP32, tag="kr")
                nc.sync.dma_start(
                    out=kr0,
                    in_=k_d[ds(GB * g, GB)].rearrange("b (p c) d -> p b c d", c=NCH),
                )
                vr0 = krawp.tile([128, GB, NCH, 128], FP32, tag="vr")
                nc.sync.dma_start(
                    out=vr0,
                    in_=v_d[ds(GB * g, GB)].rearrange("b (p c) d -> p b c d", c=NCH),
                )
                pre_kr.append(kr0)
                pre_vr.append(vr0)

            ident = constp.tile([128, 128], FP32)
            make_identity(nc, ident)
            identb = constp.tile([128, 128], BF16)
            nc.scalar.copy(identb, ident)
            identf8 = constp.tile([128, 128], F8)
            nc.scalar.copy(identf8, ident)
            qmask64 = constp.tile([128, 1], FP32)
            ones1 = constp.tile([128, 1], FP32)
            nc.vector.memset(ones1, 1.0)
            # accumulator columns: [dot 0:32 | d2 32:64 | c2 64:96 | cons 96:104]
            accum = constp.tile([128, 128], FP32)
            nc.gpsimd.memset(accum, 0.0)
            cmask = constp.tile([128, GB * M], FP32)
            nc.sync.dma_start(out=cmask, in_=cm_d)
            qmask1 = constp.tile([128, 1], FP32)
            nc.sync.dma_start(out=qmask1, in_=q1_d)
            qmaskS = constp.tile([128, 1], FP32)
            nc.sync.dma_start(out=qmaskS, in_=qs_d)
            nc.vector.tensor_scalar_mul(qmask64, qmask1, 1.0 / 64.0)

            # scps/okps junk bands are never written by matmuls; they must
            # hold finite data before the first full-tile read.  Clear every
            # rotating buffer once.
            for _ in range(2):
                sc0 = psSC.tile([128, 512], FP32, tag="scps")
                nc.vector.memset(sc0, 0.0)
                ok0 = psOK.tile([128, 256], FP32, tag="okps")
                nc.vector.memset(ok0, 0.0)

            # ---------- prologue: queries ----------
            # dense load [128 = (b2 q), 4 t, 128 d] in ONE DMA, then transpose
            # and spread into the junk-band layout on-chip.
            q_sb = constp.tile([128, 4, 128], FP32)
            nc.sync.dma_start(
                out=q_sb,
                in_=q_d.rearrange("(t b2) q d -> (b2 q) t d", t=4),
            )
            # qJ [128 d, 8 g, 128 = 4b x 32 cols] bf16, softmax scale folded in
            qJ = constp.tile([128, NG, 128], BF16)
            nc.gpsimd.memset(qJ, 0.0)
            for t in range(4):
                qtps = psSC.tile([128, 512], FP32, tag="scps")
                nc.tensor.transpose(qtps[:, 0:128], q_sb[:, t, :], ident)
                # qtps cols = b2*16+q for batches t*8..t*8+8 = groups 2t, 2t+1
                for gg in range(2):
                    g2 = 2 * t + gg
                    nc.scalar.activation(
                        out=qJ[:, g2, :].rearrange("p (j w) -> p j w", w=32)[
                            :, :, 0:Q
                        ],
                        in_=qtps[:, ds(64 * gg, 64)].rearrange(
                            "p (j w) -> p j w", w=Q
                        ),
                        func=ACTF.Copy,
                        scale=SCALE,
                    )

            # ---------- prologue: coarse-grained tensors ----------
            # kcgT [128 d, 2 t, 128 = (8 b2) x (8 m)] bf16 (no scale)
            kcg_sb = constp.tile([128, 2, 128], FP32)
            nc.sync.dma_start(
                out=kcg_sb,
                in_=kcg_d.rearrange("(t b2) m d -> (b2 m) t d", t=2),
            )
            kcgT = constp.tile([128, 2, 128], BF16)
            kcgtps = psSC.tile([128, 512], FP32, tag="scps")
            for t in range(2):
                nc.tensor.transpose(kcgtps[:, ts(t, 128)], kcg_sb[:, t, :], ident)
            nc.scalar.copy(kcgT[:], kcgtps[:, 0:256].rearrange("p (t x) -> p t x", t=2))

            # cgkvJ [32 = (4 bl x 8 m), 8 g, 2 {k|v}, 128] fp32 (okcg rhs, f32r)
            cgkvJ = constp.tile([32, NG, 2, 128], FP32)
            nc.sync.dma_start(
                out=r(cgkvJ[:, :, 0, :]),
                in_=r(kcg_d.rearrange("(g bl) m d -> (bl m) g d", bl=GB)),
            )
            nc.sync.dma_start(
                out=r(cgkvJ[:, :, 1, :]),
                in_=r(vcg_d.rearrange("(g bl) m d -> (bl m) g d", bl=GB)),
            )
            # cgkv2 [8 m, 32 b, 256 {k|v}] fp32 (cgjac1 rhs, f32r)
            cgkv2 = constp.tile([8, B, 256], FP32)
            nc.sync.dma_start(
                out=r(cgkv2[:, :, 0:128]), in_=r(kcg_d.rearrange("b m d -> m b d"))
            )
            nc.sync.dma_start(
                out=r(cgkv2[:, :, 128:256]), in_=r(vcg_d.rearrange("b m d -> m b d"))
            )

            # ---------- main loop over groups of 4 batches ----------
            for g in range(NG):
                bs = [g * GB + j for j in range(GB)]

                # raw k/v for 4 batches; 2KB-contiguous per partition line
                if g < 2:
                    kr, vr = pre_kr[g], pre_vr[g]
                else:
                    kr = krawp.tile([128, GB, NCH, 128], FP32, tag="kr")
                    nc.sync.dma_start(
                        out=kr, in_=k_d[ds(GB * g, GB)].rearrange("b (p c) d -> p b c d", c=NCH)
                    )
                    vr = krawp.tile([128, GB, NCH, 128], FP32, tag="vr")
                    nc.sync.dma_start(
                        out=vr, in_=v_d[ds(GB * g, GB)].rearrange("b (p c) d -> p b c d", c=NCH)
                    )

                # one contiguous fp8 cast per tensor per group
                kb = kvbp.tile([128, GB, NCH, 128], F8, tag="kb")
                vb = kvbp.tile([128, GB, NCH, 128], F8, tag="vb")
                if g % 2 == 0:
                    nc.scalar.copy(kb[:], kr)
                    nc.vector.tensor_copy(vb[:], vr)
                else:
                    nc.vector.tensor_copy(kb[:], kr)
                    nc.scalar.copy(vb[:], vr)

                # kT per batch via bf16 PE transpose
                kts = []
                for j in range(GB):
                    ktps = psT.tile([128, 1024], F8, tag="ktps")
                    ktv = ktps.rearrange("p (x two) -> p x two", two=2)[:, :, 0]
                    for c in range(NCH):
                        nc.tensor.transpose(
                            ktv[:, ts(c, 128)], kb[:, j, c, :], identf8
                        )
                    kt = ktp.tile([128, NCH, 128], BF16, tag="kt")
                    nc.vector.tensor_copy(kt[:], ktv.rearrange("p (c x) -> p c x", c=NCH))
                    kts.append(kt)

                # scores: batch j -> rows [32j, 32j+16)
                scps = psSC.tile([128, 512], FP32, tag="scps")
                for j in range(GB):
                    nc.tensor.matmul(
                        scps[ds(32 * j, Q), :],
                        lhsT=qJ[:, g, ds(32 * j, Q)],
                        rhs=kts[j].rearrange("p c x -> p (c x)"),
                        start=True,
                        stop=True,
                        tile_position=(0, 32 * j),
                        skip_group_check=True,
                    )

                # softmax (no max subtraction: |scores| <= ~7)
                ptil = smp.tile([128, 512], BF16, tag="ptil")
                z = smallp.tile([128, 1], FP32, tag="z")
                nc.scalar.activation(out=ptil, in_=scps, func=ACTF.Exp, accum_out=z)
                zr = smallp.tile([128, 1], FP32, tag="zr")
                nc.vector.reciprocal(zr, z)
                ptn = smp.tile([128, 512], F8, tag="ptn")
                nc.vector.tensor_scalar(
                    out=ptn, in0=ptil, scalar1=zr, scalar2=64.0,
                    op0=ALU.mult, op1=ALU.mult,
                )

                # pT via bf16 PE transpose
                ptps = psT.tile([128, 1024], F8, tag="ktps")
                ptv = ptps.rearrange("p (x two) -> p x two", two=2)[:, :, 0]
                for c in range(NCH):
                    nc.tensor.transpose(
                        ptv[:, ts(c, 128)], ptn[:, ts(c, 128)], identf8
                    )
                pT = smp.tile([128, NCH, 128], F8, tag="pT")
                nc.vector.tensor_copy(pT[:], ptv.rearrange("p (c x) -> p c x", c=NCH))

                # c = sum_q p (valid q rows only), scaled by SCALE
                c_t = smallp.tile([128, NCH, GB], FP32, tag="c_t")
                nc.vector.tensor_reduce(
                    out=c_t,
                    in_=pT.rearrange("p c (j w) -> p c j w", j=GB)[:, :, :, 0:Q],
                    axis=AX.X,
                    op=ALU.add,
                )
                nc.gpsimd.tensor_scalar_mul(c_t, c_t, SCALE)

                # out/kbar: per batch 4 chunk-matmuls into rows [32j, 32j+16)
                # DoubleRow dst must sit at partition 0, so out/kbar use
                # plain fp8 chunk matmuls (rate 1).
                okps = psOK.tile([128, 256], FP32, tag="okps")
                for j in range(GB):
                    for h, rhs_t in ((0, kb), (1, vb)):
                        for c in range(NCH):
                            nc.tensor.matmul(
                                okps[ds(32 * j, Q), ds(128 * h, 128)],
                                lhsT=pT[:, c, ds(32 * j, Q)],
                                rhs=rhs_t[:, j, c, :],
                                start=(c == 0),
                                stop=(c == NCH - 1),
                                tile_position=(0, 32 * j),
                                skip_group_check=True,
                            )
                # okb [128, 256] bf16 = [-s*kbar | out], junk rows zeroed
                okb = okp.tile([128, 256], BF16, tag="okb")
                nc.scalar.activation(
                    out=okb[:, 0:128], in_=okps[:, 0:128], func=ACTF.Copy, scale=qmaskS
                )
                nc.scalar.activation(
                    out=okb[:, 128:256], in_=okps[:, 128:256], func=ACTF.Copy,
                    scale=qmask64,
                )

                # ---- coarse-grained chain (batched over the group) ----
                cgs = psOK.tile([128, 64], FP32, tag="okps")
                nc.tensor.matmul(
                    cgs[:, 0 : GB * M],
                    lhsT=qJ[:, g, :],
                    rhs=kcgT[:, g // 4, ds(8 * ((GB * g) % 16), GB * M)],
                    start=True,
                    stop=True,
                    skip_group_check=True,
                )
                ecg = scrp.tile([128, GB * M], FP32, tag="ecg")
                nc.scalar.activation(out=ecg, in_=cgs[:, 0 : GB * M], func=ACTF.Exp)
                em = scrp.tile([128, GB * M], FP32, tag="em")
                zcg = smallp.tile([128, 1], FP32, tag="zcg")
                nc.vector.scalar_tensor_tensor(
                    out=em, in0=ecg, scalar=1.0, in1=cmask,
                    op0=ALU.mult, op1=ALU.mult, accum_out=zcg,
                )
                nc.vector.tensor_scalar_add(zcg, zcg, 1e-20)
                zcgr = smallp.tile([128, 1], FP32, tag="zcgr")
                nc.vector.reciprocal(zcgr, zcg)
                pcgn = scrp.tile([128, GB * M], FP32, tag="pcgn")
                nc.scalar.activation(
                    out=pcgn, in_=em, func=ACTF.Copy, scale=zcgr
                )

                # pcgT [32 = (4 bl x 8 m), 128 = 4b x 32 q-cols] (block-diagonal)
                pcgps = psOK.tile([32, 128], FP32, tag="okps")
                nc.tensor.transpose(pcgps, pcgn, ident)
                pcgTs = scrp.tile([32, 128], FP32, tag="pcgTs")
                nc.scalar.copy(r(pcgTs[:]), pcgps)

                # c_cg per (bl, m): block-diagonal rows sum over own q window
                ccg = smallp.tile([32, 1], FP32, tag="ccg")
                nc.vector.tensor_reduce(out=ccg, in_=pcgTs, axis=AX.X, op=ALU.add)
                vccgJ = scrp.tile([32, 128], FP32, tag="vccgJ")
                nc.vector.tensor_scalar(
                    out=r(vccgJ[:]),
                    in0=cgkvJ[:, g, 1, :],
                    scalar1=ccg,
                    scalar2=SCALE,
                    op0=ALU.mult,
                    op1=ALU.mult,
                )

                # restage vccgJ per batch at partition base 0 (4 safe
                # partition-slice DMAs; a single partition-splitting rearrange
                # DMA silently miscopies)
                vccg4 = smallp.tile([8, GB, 128], FP32, tag="vccg4")
                for bl in range(GB):
                    nc.sync.dma_start(
                        out=r(vccg4[:, bl, :]),
                        in_=r(vccgJ[ds(8 * bl, 8), :]),
                    )

                # cg out/kbar for all 4 batches: one f32r matmul
                okcgps = psOK.tile([128, 256], FP32, tag="okps")
                nc.tensor.matmul(
                    okcgps,
                    lhsT=r(pcgTs),
                    rhs=r(cgkvJ[:, g, :, :].rearrange("p a x -> p (a x)")),
                    start=True,
                    stop=True,
                    skip_group_check=True,
                )
                okcg = okp.tile([128, 256], BF16, tag="okcg")
                nc.scalar.activation(
                    out=okcg[:, 0:128], in_=okcgps[:, 0:128], func=ACTF.Copy,
                    scale=qmaskS,
                )
                nc.scalar.activation(
                    out=okcg[:, 128:256], in_=okcgps[:, 128:256], func=ACTF.Copy,
                    scale=qmask1,
                )

                # consistency for the whole group (junk rows are zero)
                dif = scrp.tile([128, 128], BF16, tag="dif")
                nc.vector.tensor_sub(dif, okb[:, 128:256], okcg[:, 128:256])
                scc = scrp.tile([128, 128], BF16, tag="scc")
                nc.vector.scalar_tensor_tensor(
                    out=scc, in0=dif, scalar=1.0, in1=dif,
                    op0=ALU.mult, op1=ALU.mult,
                    accum_out=accum[:, ds(96 + g, 1)],
                )

                if debug_dump and g == 0:
                    okraw = scrp.tile([128, 256], FP32, tag="okraw")
                    nc.vector.tensor_copy(okraw, okps)
                    nc.sync.dma_start(out=dbg_okraw, in_=okraw)
                    cgsb = scrp.tile([128, 32], FP32, tag="cgsb")
                    nc.vector.tensor_copy(cgsb, cgs[:, 0 : GB * M])
                    nc.sync.dma_start(out=dbg_cgs, in_=cgsb)
                    nc.sync.dma_start(out=dbg_em, in_=em)
                    nc.sync.dma_start(out=dbg_pt, in_=ptn)
                    nc.sync.dma_start(out=dbg_okb, in_=okb)
                    nc.sync.dma_start(out=dbg_okcg, in_=okcg)
                    nc.sync.dma_start(out=dbg_pcg, in_=pcgTs)

                # ---- per-batch jacobians ----
                jall = scrp.tile([128, GB, 256], BF16, tag="jall")
                for j, b in enumerate(bs):
                    vc = smallp.tile([128, NCH, 128], F8, tag="vc")
                    veng = nc.gpsimd if j % 2 == 0 else nc.vector
                    veng.tensor_tensor(
                        out=vc,
                        in0=vb[:, j, :, :],
                        in1=c_t[:, :, ds(j, 1)].broadcast_to([128, NCH, 128]),
                        op=ALU.mult,
                    )

                    jp = psJ.tile([128, 512], FP32, tag="jd")
                    # cg jac: [128:256] real (k half), [256:384] waste (v half)
                    nc.tensor.matmul(
                        jp[:, 128:384],
                        lhsT=r(vccg4[:, j, :]),
                        rhs=r(cgkv2[:, b, :]),
                        start=True,
                        stop=False,
                        skip_group_check=True,
                    )
                    nc.tensor.matmul(
                        jp[:, 128:256],
                        lhsT=okcg[ds(32 * j, Q), 128:256],
                        rhs=okcg[ds(32 * j, Q), 0:128],
                        start=False,
                        stop=True,
                        tile_position=(32 * j, 0),
                        skip_group_check=True,
                    )
                    # dense jac (x64): fp8 DoubleRow chunk-pairs + out^T(-64 s kbar)
                    for cp in range(NCH // 2):
                        nc.tensor.matmul(
                            jp[:, 0:128],
                            lhsT=vc[:, ds(2 * cp, 2), :],
                            rhs=kb[:, j, ds(2 * cp, 2), :],
                            start=(cp == 0),
                            stop=False,
                            perf_mode=DR,
                            skip_group_check=True,
                        )
                    nc.tensor.matmul(
                        jp[:, 0:128],
                        lhsT=okb[ds(32 * j, Q), 128:256],
                        rhs=okb[ds(32 * j, Q), 0:128],
                        start=False,
                        stop=True,
                        tile_position=(32 * j, 0),
                        skip_group_check=True,
                    )

                    if debug_dump and g == 0 and j == 0:
                        jbounce = scrp.tile([128, 256], FP32, tag="jbounce")
                        nc.vector.tensor_copy(jbounce, jp[:, 0:256])
                        nc.sync.dma_start(out=dbg_jp, in_=jbounce)

                    nc.scalar.copy(jall[:, j, :], jp[:, 0:256])

                # group-level dot/norm reductions off SBUF
                pr = scrp.tile([128, GB, 128], BF16, tag="pr")
                nc.vector.tensor_tensor(
                    out=pr, in0=jall[:, :, 0:128], in1=jall[:, :, 128:256],
                    op=ALU.mult,
                )
                nc.vector.tensor_reduce(
                    out=accum[:, ds(12 * g, GB)], in_=pr, axis=AX.X, op=ALU.add
                )
                sd = scrp.tile([128, GB, 128], BF16, tag="sd")
                nc.scalar.activation(
                    out=sd, in_=jall[:, :, 0:128], func=ACTF.Square
                )
                nc.vector.tensor_reduce(
                    out=accum[:, ds(12 * g + 4, GB)], in_=sd, axis=AX.X,
                    op=ALU.add,
                )
                sc2 = scrp.tile([128, GB, 128], BF16, tag="sc2")
                nc.scalar.activation(
                    out=sc2, in_=jall[:, :, 128:256], func=ACTF.Square
                )
                nc.vector.tensor_reduce(
                    out=accum[:, ds(12 * g + 8, GB)], in_=sc2, axis=AX.X,
                    op=ALU.add,
                )

            # ---------- final reduction ----------
            if debug_dump:
                nc.sync.dma_start(out=dbg_acc, in_=accum)
            rps = psJ.tile([1, 128], FP32, tag="jd")
            nc.tensor.matmul(
                rps, lhsT=ones1, rhs=accum, start=True, stop=True,
                skip_group_check=True,
            )
            row = constp.tile([1, 128], FP32)
            nc.scalar.copy(row, rps)
            rw = row[:, 0:96].rearrange("o (g t f) -> o g t f", t=3, f=GB)
            f1 = constp.tile([1, 32], FP32)
            f1v = f1.rearrange("o (g f) -> o g f", f=GB)
            nc.vector.tensor_tensor(
                out=f1v, in0=rw[:, :, 1, :], in1=rw[:, :, 2, :], op=ALU.mult
            )
            nc.scalar.activation(out=f1, in_=f1, func=ACTF.Sqrt)
            nc.vector.tensor_scalar_add(f1, f1, EPS)
            f2 = constp.tile([1, 32], FP32)
            f2v = f2.rearrange("o (g f) -> o g f", f=GB)
            nc.vector.reciprocal(f2, f1)
            nc.vector.tensor_tensor(
                out=f2v, in0=rw[:, :, 0, :], in1=f2v, op=ALU.mult
            )
            csum = constp.tile([1, 1], FP32)
            nc.vector.tensor_reduce(out=csum, in_=f2, axis=AX.X, op=ALU.add)
            msum = constp.tile([1, 1], FP32)
            nc.vector.tensor_reduce(
                out=msum, in_=row[:, 96:128], axis=AX.X, op=ALU.add
            )
            part = constp.tile([1, 2], FP32)
            nc.vector.tensor_scalar(
                out=part[:, 0:1], in0=csum, scalar1=-1.0, scalar2=float(B),
                op0=ALU.mult, op1=ALU.add,
            )
            nc.vector.tensor_scalar_mul(part[:, 1:2], msum, 1.0 / QD)
            nc.sync.dma_start(out=out_d, in_=part)

    return nc


_NC_CACHE = {}


def _get_nc():
    if "nc" not in _NC_CACHE:
        _install_drain_fix()
        nc = build_nc()
        _split_waits(nc)
        _NC_CACHE["nc"] = nc
    return _NC_CACHE["nc"]


def _split_waits(nc):
    """This walrus accepts only one sync-wait per instruction; move extras
    onto same-engine NoOps inserted just before."""
    from concourse import mybir
    from bass_rust import SyncInfo

    for f in nc.m.functions:
        for blk in f.blocks:
            insts = list(blk.instructions)
            out = []
            for inst in insts:
                si = inst.sync_info
                waits = list(si.on_wait) if si is not None else []
                if len(waits) > 1:
                    for wi, w in enumerate(waits[:-1]):
                        nop = mybir.InstNoOp(name=f"{inst.name}-wsplit{wi}")
                        nop.engine = inst.engine
                        nop.sync_info = SyncInfo(on_wait=[w], on_update=[])
                        out.append(nop)
                    inst.sync_info = SyncInfo(
                        on_wait=[waits[-1]], on_update=list(si.on_update)
                    )
                out.append(inst)
            blk.instructions = out


N_CORES = 8


def _in_maps(queries, keys, values, k_cg, v_cg):
    consts = host_consts()
    nb = queries.shape[0]
    sh = nb // N_CORES
    return [
        {
            "queries": queries[i * sh : (i + 1) * sh],
            "keys": keys[i * sh : (i + 1) * sh],
            "values": values[i * sh : (i + 1) * sh],
            "k_cg": k_cg[i * sh : (i + 1) * sh],
            "v_cg": v_cg[i * sh : (i + 1) * sh],
            **consts,
        }
        for i in range(N_CORES)
    ]


def kernel(queries, keys, values, k_cg, v_cg):
    from concourse.bass_utils import run_bass_kernel_spmd

    queries = np.ascontiguousarray(np.asarray(queries, dtype=np.float32))
    keys = np.ascontiguousarray(np.asarray(keys, dtype=np.float32))
    values = np.ascontiguousarray(np.asarray(values, dtype=np.float32))
    k_cg = np.ascontiguousarray(np.asarray(k_cg, dtype=np.float32))
    v_cg = np.ascontiguousarray(np.asarray(v_cg, dtype=np.float32))

    nb = queries.shape[0]
    in_maps = _in_maps(queries, keys, values, k_cg, v_cg)
    nc = _get_nc()
    res = run_bass_kernel_spmd(nc, in_maps, core_ids=list(range(N_CORES)))
    total = 0.0
    for i in range(N_CORES):
        part = res.results[i]["out"]
        total += float(part[0, 0]) + float(part[0, 1])
    return np.float32(total / nb)



# revision 23
# speedup vs baseline: 1.1893x; 1.1893x over previous
"""Trainium2 Bass kernel for nn_ForceMatchingLoss (batch-data-parallel over 8 NeuronCores).

Full inputs (B=256) are sharded along the batch dimension: core i computes
batches [32*i, 32*i+32) and returns [sum_b fd_b, sum_b cons_b]; the host
sums the 8 partials and divides by 256 (the loss is a batch mean, so the
"all-reduce" is a trivial host-side sum of 8 scalars).

v3: fp8 DMA-cast loads (SWDGE casts fp32->fp8 in flight, no on-chip cast
ops), out/kbar computed in the d-major direction with fp8 DoubleRow
matmuls (M=128 instead of M=16: 4096 -> ~400 PE cycles/group) then
transposed back in two batched PE transposes, coarse-grained chain in a
32-padded partition layout (kills the 4-per-group partition-restage DMAs
and runs the cg jacobian in fp8), and per-batch jacobian reductions fused
into single accum ops reading PSUM directly.
"""

import numpy as np


# ---------------------------------------------------------------------------
# Workaround for this walrus build: CTRL-type instructions (Drain) only accept
# a single sync-wait; TileContext's tail drain aggregates one wait per logical
# processor.  Split the waits across a chain of drains.
# ---------------------------------------------------------------------------
def _install_drain_fix():
    import concourse.tile as tile
    from bass_rust import ScopedClock, SyncInfo

    if getattr(tile.TileContext, "_drain_fix_installed", False):
        return

    def _drain_and_barrier(self, tick_clock, wait_clock):
        drain_inst = self.nc.sync.drain()
        wait_clock.add_sem_waits(
            drain_inst.ins, ScopedClock({None: tick_clock.global_clock})
        )
        si = drain_inst.ins.sync_info
        waits = list(si.on_wait) if si is not None else []
        if len(waits) > 1:
            drain_inst.ins.sync_info = SyncInfo(
                on_wait=waits[:1], on_update=list(si.on_update)
            )
            for i in range(1, len(waits)):
                d = self.nc.sync.drain()
                d.ins.sync_info = SyncInfo(on_wait=waits[i : i + 1], on_update=[])

        self.nc.all_engine_barrier()
        popped = self.nc._tile_sem_poison_stack.pop()
        assert popped is self._sem_poison
        self.nc.clear_and_free_semaphores(list(self.sems.allocated().values()))
        self.nc.all_engine_barrier()

    tile.TileContext._drain_and_barrier = _drain_and_barrier
    tile.TileContext._drain_fix_installed = True


import concourse.bass as bass
import concourse.tile as tile
from concourse import mybir
from concourse.bass import ds, ts
from concourse.masks import make_identity

FP32 = mybir.dt.float32
BF16 = mybir.dt.bfloat16
F32R = mybir.dt.float32r
F8 = mybir.dt.float8e4
DR = mybir.MatmulPerfMode.DoubleRow
AX = mybir.AxisListType
ALU = mybir.AluOpType
ACTF = mybir.ActivationFunctionType

B = 32          # batches per core
Q = 16
S = 512
M = 8
D = 128
NCH = 4         # s chunks of 128
GB = 4          # batches per group (32-row bands in the scores psum tile)
NG = B // GB    # 8 groups
SCALE = float(D) ** -0.5
EPS = 1e-8
QD = float(Q * D)
PREFETCH = 3    # groups of k/v loads in flight ahead of compute


def r(ap):
    return ap.bitcast(F32R)


def host_consts():
    """Constant mask tensors supplied as extra kernel inputs."""
    # cmask [128, 32]: block-diagonal valid mask for the batched cg scores.
    # row = 32*j + qi (qi valid when < 16), col = 8*j' + m; 1 iff j==j', qi<16.
    cm = np.zeros((128, GB * M), dtype=np.float32)
    for j in range(GB):
        cm[32 * j : 32 * j + Q, M * j : M * j + M] = 1.0
    # qmask1 / qmaskS [128,1]: per-partition valid-row masks for the okb/okcg
    # psum->sbuf copies (junk rows zeroed; kbar half also folds -SCALE).
    q1 = np.zeros((128, 1), dtype=np.float32)
    qs = np.zeros((128, 1), dtype=np.float32)
    for j in range(GB):
        q1[32 * j : 32 * j + Q] = 1.0
        qs[32 * j : 32 * j + Q] = -SCALE
    return {"cmask": cm, "qmask1": q1, "qmaskS": qs}


def build_nc():
    nc = bass.Bass("TRN2", target_bir_lowering=False, debug=False)
    q_d = nc.dram_tensor("queries", [B, Q, D], FP32, kind="ExternalInput").ap()
    k_d = nc.dram_tensor("keys", [B, S, D], FP32, kind="ExternalInput").ap()
    v_d = nc.dram_tensor("values", [B, S, D], FP32, kind="ExternalInput").ap()
    kcg_d = nc.dram_tensor("k_cg", [B, M, D], FP32, kind="ExternalInput").ap()
    vcg_d = nc.dram_tensor("v_cg", [B, M, D], FP32, kind="ExternalInput").ap()
    cm_d = nc.dram_tensor("cmask", [128, GB * M], FP32, kind="ExternalInput").ap()
    q1_d = nc.dram_tensor("qmask1", [128, 1], FP32, kind="ExternalInput").ap()
    qs_d = nc.dram_tensor("qmaskS", [128, 1], FP32, kind="ExternalInput").ap()
    out_d = nc.dram_tensor("out", [1, 2], FP32, kind="ExternalOutput").ap()

    with tile.TileContext(nc) as tc:
        with (
            tc.tile_pool(name="const", bufs=1) as constp,
            tc.tile_pool(name="kvb", bufs=1 + PREFETCH) as kvbp,
            tc.tile_pool(name="kt", bufs=5) as ktp,
            tc.tile_pool(name="sm", bufs=3) as smp,
            tc.tile_pool(name="ptp", bufs=3) as ptp,
            tc.tile_pool(name="vcp", bufs=3) as vcp,
            tc.tile_pool(name="small", bufs=4) as smallp,
            tc.tile_pool(name="ok", bufs=3) as okp,
            tc.tile_pool(name="scr", bufs=4) as scrp,
            tc.tile_pool(name="psSC", bufs=2, space="PSUM") as psSC,
            tc.tile_pool(name="psT", bufs=2, space="PSUM") as psT,
            tc.tile_pool(name="psCG", bufs=2, space="PSUM") as psCG,
            tc.tile_pool(name="psJ", bufs=2, space="PSUM") as psJ,
        ):
            # ---------- k/v prefetch (SWDGE DMA-cast fp32 -> fp8) ----------
            kvq = {}

            def load_group(gg):
                kb = kvbp.tile([128, GB, NCH, 128], F8, tag="kb")
                nc.gpsimd.dma_start(
                    out=kb,
                    in_=k_d[ds(GB * gg, GB)].rearrange(
                        "b (p c) d -> p b c d", c=NCH
                    ),
                )
                vb = kvbp.tile([128, GB, NCH, 128], F8, tag="vb")
                nc.gpsimd.dma_start(
                    out=vb,
                    in_=v_d[ds(GB * gg, GB)].rearrange(
                        "b (p c) d -> p b c d", c=NCH
                    ),
                )
                kvq[gg] = (kb, vb)

            for gg in range(min(PREFETCH, NG)):
                load_group(gg)

            # ---------- constants ----------
            ident = constp.tile([128, 128], FP32)
            make_identity(nc, ident)
            identb = constp.tile([128, 128], BF16)
            nc.scalar.copy(identb, ident)
            identf8 = constp.tile([128, 128], F8)
            nc.scalar.copy(identf8, ident)
            ones1 = constp.tile([128, 1], FP32)
            nc.vector.memset(ones1, 1.0)
            # accumulator columns: per group g, [dot 12g:12g+4 | d2 +4 | c2 +8];
            # cons at 96+g
            accum = constp.tile([128, 128], FP32)
            nc.gpsimd.memset(accum, 0.0)
            cmask = constp.tile([128, GB * M], FP32)
            nc.sync.dma_start(out=cmask, in_=cm_d)
            qmask1 = constp.tile([128, 1], FP32)
            nc.sync.dma_start(out=qmask1, in_=q1_d)
            qmaskS = constp.tile([128, 1], FP32)
            nc.sync.dma_start(out=qmaskS, in_=qs_d)
            qmask64 = constp.tile([128, 1], FP32)
            nc.vector.tensor_scalar_mul(qmask64, qmask1, 1.0 / 64.0)

            # scps junk bands are never written by the M=16 score matmuls;
            # they must hold finite data before the full-tile exp (cols
            # 0:256 get overwritten by the okT reuse each group, which keeps
            # them finite thereafter).  Clear every rotating buffer once.
            for _ in range(2):
                sc0 = psSC.tile([128, 512], FP32, tag="scps")
                nc.vector.memset(sc0, 0.0)

            # ---------- prologue: queries ----------
            # dense load [128 = (b2 q), 4 t, 128 d] in ONE DMA, then transpose
            # and spread into the junk-band layout on-chip.
            q_sb = constp.tile([128, 4, 128], FP32)
            nc.sync.dma_start(
                out=q_sb,
                in_=q_d.rearrange("(t b2) q d -> (b2 q) t d", t=4),
            )
            # qJ [128 d, 8 g, 128 = 4b x 32 cols] bf16, softmax scale folded in
            qJ = constp.tile([128, NG, 128], BF16)
            nc.gpsimd.memset(qJ, 0.0)
            for t in range(4):
                qtps = psSC.tile([128, 512], FP32, tag="scps")
                nc.tensor.transpose(qtps[:, 0:128], q_sb[:, t, :], ident)
                # qtps cols = b2*16+q for batches t*8..t*8+8 = groups 2t, 2t+1
                for gg in range(2):
                    g2 = 2 * t + gg
                    nc.scalar.activation(
                        out=qJ[:, g2, :].rearrange("p (j w) -> p j w", w=32)[
                            :, :, 0:Q
                        ],
                        in_=qtps[:, ds(64 * gg, 64)].rearrange(
                            "p (j w) -> p j w", w=Q
                        ),
                        func=ACTF.Copy,
                        scale=SCALE,
                    )

            # ---------- prologue: coarse-grained tensors ----------
            # kcgT [128 d, 2 t, 128 = (8 b2) x (8 m)] bf16 (no scale)
            kcg_sb = constp.tile([128, 2, 128], FP32)
            nc.sync.dma_start(
                out=kcg_sb,
                in_=kcg_d.rearrange("(t b2) m d -> (b2 m) t d", t=2),
            )
            kcgT = constp.tile([128, 2, 128], BF16)
            kcgtps = psSC.tile([128, 512], FP32, tag="scps")
            for t in range(2):
                nc.tensor.transpose(kcgtps[:, ts(t, 128)], kcg_sb[:, t, :], ident)
            nc.scalar.copy(kcgT[:], kcgtps[:, 0:256].rearrange("p (t x) -> p t x", t=2))

            # Padded cg layout: partition row 32*bl + m holds (batch 4g+bl,
            # coarse key m); junk partitions stay zero so the block-diagonal
            # okcg matmul and the per-batch jac1cg slices read clean data.
            cgkvJ32 = constp.tile([128, NG, 2, 128], FP32)
            nc.gpsimd.memset(cgkvJ32, 0.0)
            kcg8 = constp.tile([128, NG, 128], F8)
            for bl in range(GB):
                src_k = kcg_d.rearrange("(g bl) m d -> bl m g d", bl=GB)[ds(bl, 1)]
                src_v = vcg_d.rearrange("(g bl) m d -> bl m g d", bl=GB)[ds(bl, 1)]
                nc.sync.dma_start(
                    out=r(cgkvJ32[ds(32 * bl, M), :, 0, :]), in_=r(src_k)
                )
                nc.sync.dma_start(
                    out=r(cgkvJ32[ds(32 * bl, M), :, 1, :]), in_=r(src_v)
                )
                nc.gpsimd.dma_start(out=kcg8[ds(32 * bl, M), :, :], in_=src_k)

            # persistent padded pcg tile: valid cols 32*bl + m, junk cols
            # zeroed once and never written again.
            pcgn128 = constp.tile([128, 128], FP32)
            nc.gpsimd.memset(pcgn128, 0.0)

            # ---------- main loop over groups of 4 batches ----------
            for g in range(NG):
                if g + PREFETCH < NG:
                    load_group(g + PREFETCH)
                kb, vb = kvq.pop(g)

                # kT per batch via fp8 PE transpose; copies spread over
                # Act/DVE/Pool
                kts = []
                for j in range(GB):
                    ktps = psT.tile([128, 1024], F8, tag="ktps")
                    ktv = ktps.rearrange("p (x two) -> p x two", two=2)[:, :, 0]
                    for c in range(NCH):
                        nc.tensor.transpose(
                            ktv[:, ts(c, 128)], kb[:, j, c, :], identf8
                        )
                    kt = ktp.tile([128, NCH, 128], BF16, tag="kt")
                    ktv_r = ktv.rearrange("p (c x) -> p c x", c=NCH)
                    if j % 2 == 0:
                        nc.vector.tensor_copy(kt[:], ktv_r)
                    else:
                        nc.scalar.copy(kt[:], ktv_r)
                    kts.append(kt)

                # scores: batch j -> rows [32j, 32j+16)
                scps = psSC.tile([128, 512], FP32, tag="scps")
                for j in range(GB):
                    nc.tensor.matmul(
                        scps[ds(32 * j, Q), :],
                        lhsT=qJ[:, g, ds(32 * j, Q)],
                        rhs=kts[j].rearrange("p c x -> p (c x)"),
                        start=True,
                        stop=True,
                        tile_position=(0, 32 * j),
                        skip_group_check=True,
                    )

                # cg scores early so the long small-op cg chain overlaps the
                # dense chain.  One merged psum tile per group holds the cg
                # scores, the okcg matmul output, and the pcg transpose:
                # cols [0:64 cgs | 64:320 okcgps | 320:448 pcgps].
                cgall = psCG.tile([128, 512], FP32, tag="cgall")
                cgs = cgall[:, 0:64]
                okcgps = cgall[:, 64:320]
                pcgps = cgall[:, 320:448]
                nc.tensor.matmul(
                    cgs[:, 0 : GB * M],
                    lhsT=qJ[:, g, :],
                    rhs=kcgT[:, g // 4, ds(8 * ((GB * g) % 16), GB * M)],
                    start=True,
                    stop=True,
                    skip_group_check=True,
                )

                # softmax (no max subtraction: |scores| <= ~7)
                ptil = smp.tile([128, 512], BF16, tag="ptil")
                z = smallp.tile([128, 1], FP32, tag="z")
                nc.scalar.activation(out=ptil, in_=scps, func=ACTF.Exp, accum_out=z)
                zr = smallp.tile([128, 1], FP32, tag="zr")
                nc.vector.reciprocal(zr, z)
                ptn = smp.tile([128, 512], F8, tag="ptn")
                nc.vector.tensor_scalar(
                    out=ptn, in0=ptil, scalar1=zr, scalar2=64.0,
                    op0=ALU.mult, op1=ALU.mult,
                )

                # cg softmax chain (Act/DVE small ops)
                ecg = scrp.tile([128, GB * M], FP32, tag="ecg")
                nc.scalar.activation(out=ecg, in_=cgs[:, 0 : GB * M], func=ACTF.Exp)
                em = scrp.tile([128, GB * M], FP32, tag="em")
                zcg = smallp.tile([128, 1], FP32, tag="zcg")
                nc.vector.scalar_tensor_tensor(
                    out=em, in0=ecg, scalar=1.0, in1=cmask,
                    op0=ALU.mult, op1=ALU.mult, accum_out=zcg,
                )
                nc.vector.tensor_scalar_add(zcg, zcg, 1e-20)
                zcgr = smallp.tile([128, 1], FP32, tag="zcgr")
                nc.vector.reciprocal(zcgr, zcg)
                nc.scalar.activation(
                    out=pcgn128.rearrange("p (bl w) -> p bl w", w=32)[:, :, 0:M],
                    in_=em.rearrange("p (bl m) -> p bl m", m=M),
                    func=ACTF.Copy,
                    scale=zcgr,
                )

                # pT via fp8 PE transpose
                ptps = psT.tile([128, 1024], F8, tag="ktps")
                ptv = ptps.rearrange("p (x two) -> p x two", two=2)[:, :, 0]
                for c in range(NCH):
                    nc.tensor.transpose(
                        ptv[:, ts(c, 128)], ptn[:, ts(c, 128)], identf8
                    )
                pT = ptp.tile([128, NCH, 128], F8, tag="pT")
                nc.vector.tensor_copy(pT[:], ptv.rearrange("p (c x) -> p c x", c=NCH))

                # outT/kbarT: fp8 DR, M=128 (d on partitions), 16 tiny
                # matmuls.  The score psum tile is dead after exp, so its
                # cols [0:128] host the DR outputs and [128:256] (bitcast to
                # bf16) host the transposed-back okKO.
                okTps = scps[:, 0:128].rearrange(
                    "p (h j q) -> p h j q", h=2, j=GB
                )
                for j in range(GB):
                    for h, src in ((0, kb), (1, vb)):
                        for cp in range(2):
                            nc.tensor.matmul(
                                okTps[:, h, j, :],
                                lhsT=src[:, j, ds(2 * cp, 2), :],
                                rhs=pT[:, ds(2 * cp, 2), ds(32 * j, Q)],
                                start=(cp == 0),
                                stop=(cp == 1),
                                perf_mode=DR,
                                skip_group_check=True,
                            )

                # c = sum_q p (valid q rows only); SCALE folded into vc below
                c_t = smallp.tile([128, NCH, GB], FP32, tag="c_t")
                nc.vector.tensor_reduce(
                    out=c_t,
                    in_=pT.rearrange("p c (j w) -> p c j w", j=GB)[:, :, :, 0:Q],
                    axis=AX.X,
                    op=ALU.add,
                )
                nc.gpsimd.tensor_scalar_mul(c_t, c_t, SCALE)
                vcs = []
                for j in range(GB):
                    vc = vcp.tile([128, NCH, 128], F8, tag="vc")
                    veng = nc.vector if j % 2 == 0 else nc.gpsimd
                    veng.tensor_tensor(
                        out=vc,
                        in0=vb[:, j, :, :],
                        in1=c_t[:, :, ds(j, 1)].broadcast_to([128, NCH, 128]),
                        op=ALU.mult,
                    )
                    vcs.append(vc)

                # transpose outT/kbarT back to the q-banded layout: staging
                # copy (junk cols zero) then one batched transpose per half.
                okTs = okp.tile([128, 2, GB, 32], BF16, tag="okTs")
                nc.gpsimd.memset(okTs, 0.0)
                nc.vector.tensor_copy(okTs[:, :, :, 0:Q], okTps)
                okKO = scps[:, 128:256].bitcast(BF16).rearrange(
                    "p (h x) -> p h x", h=2
                )
                nc.tensor.transpose(okKO[:, 0, :], okTs[:, 0, :, :], identb)
                nc.tensor.transpose(okKO[:, 1, :], okTs[:, 1, :, :], identb)
                # okb [128, 256] bf16 = [-64*s*kbar | out], junk rows zeroed
                okb = okp.tile([128, 256], BF16, tag="okb")
                nc.scalar.activation(
                    out=okb[:, 0:128], in_=okKO[:, 0, :], func=ACTF.Copy,
                    scale=qmaskS,
                )
                nc.scalar.activation(
                    out=okb[:, 128:256], in_=okKO[:, 1, :], func=ACTF.Copy,
                    scale=qmask64,
                )

                # ---- coarse-grained out/kbar (padded block-diagonal) ----
                nc.tensor.transpose(pcgps, pcgn128, ident)
                pcgTs = scrp.tile([128, 128], FP32, tag="pcgTs")
                nc.scalar.copy(r(pcgTs[:]), pcgps)
                ccg32 = smallp.tile([128, 1], FP32, tag="ccg")
                nc.vector.tensor_reduce(out=ccg32, in_=pcgTs, axis=AX.X, op=ALU.add)
                vccg32 = scrp.tile([128, 128], F8, tag="vccg")
                nc.vector.tensor_scalar(
                    out=vccg32,
                    in0=cgkvJ32[:, g, 1, :],
                    scalar1=ccg32,
                    scalar2=SCALE,
                    op0=ALU.mult,
                    op1=ALU.mult,
                )
                nc.tensor.matmul(
                    okcgps,
                    lhsT=r(pcgTs),
                    rhs=r(cgkvJ32[:, g, :, :].rearrange("p a x -> p (a x)")),
                    start=True,
                    stop=True,
                    skip_group_check=True,
                )
                okcg = okp.tile([128, 256], BF16, tag="okcg")
                nc.scalar.activation(
                    out=okcg[:, 0:128], in_=okcgps[:, 0:128], func=ACTF.Copy,
                    scale=qmaskS,
                )
                nc.scalar.activation(
                    out=okcg[:, 128:256], in_=okcgps[:, 128:256], func=ACTF.Copy,
                    scale=qmask1,
                )

                # consistency for the whole group (junk rows are zero)
                dif = scrp.tile([128, 128], BF16, tag="dif")
                nc.gpsimd.tensor_sub(dif, okb[:, 128:256], okcg[:, 128:256])
                scc = scrp.tile([128, 128], BF16, tag="scc")
                nc.vector.scalar_tensor_tensor(
                    out=scc, in0=dif, scalar=1.0, in1=dif,
                    op0=ALU.mult, op1=ALU.mult,
                    accum_out=accum[:, ds(96 + g, 1)],
                )

                # ---- per-batch jacobians (2 per psum tile) ----
                jall = scrp.tile([128, GB, 256], BF16, tag="jall")
                for pair in range(2):
                    jp = psJ.tile([128, 2, 256], FP32, tag="jd")
                    for jj in range(2):
                        j = 2 * pair + jj
                        # dense jac (x64): fp8 DR chunk-pairs + out^T(-64 s kbar)
                        for cp in range(2):
                            nc.tensor.matmul(
                                jp[:, jj, 0:128],
                                lhsT=vcs[j][:, ds(2 * cp, 2), :],
                                rhs=kb[:, j, ds(2 * cp, 2), :],
                                start=(cp == 0),
                                stop=False,
                                perf_mode=DR,
                                skip_group_check=True,
                            )
                        nc.tensor.matmul(
                            jp[:, jj, 0:128],
                            lhsT=okb[ds(32 * j, Q), 128:256],
                            rhs=okb[ds(32 * j, Q), 0:128],
                            start=False,
                            stop=True,
                            tile_position=(32 * j, 0),
                            skip_group_check=True,
                        )
                        # cg jac: fp8 jac1 + bf16 jac2, same psum region
                        nc.tensor.matmul(
                            jp[:, jj, 128:256],
                            lhsT=vccg32[ds(32 * j, M), :],
                            rhs=kcg8[ds(32 * j, M), g, :],
                            start=True,
                            stop=False,
                            tile_position=(32 * j, 0),
                            skip_group_check=True,
                        )
                        nc.tensor.matmul(
                            jp[:, jj, 128:256],
                            lhsT=okcg[ds(32 * j, Q), 128:256],
                            rhs=okcg[ds(32 * j, Q), 0:128],
                            start=False,
                            stop=True,
                            tile_position=(32 * j, 0),
                            skip_group_check=True,
                        )
                    # bounce the pair to SBUF bf16 for the reductions
                    nc.scalar.copy(jall[:, ds(2 * pair, 2), :], jp)

                # group-level dot/norm reductions off SBUF
                pr = scrp.tile([128, GB, 128], BF16, tag="pr")
                nc.vector.tensor_tensor(
                    out=pr, in0=jall[:, :, 0:128], in1=jall[:, :, 128:256],
                    op=ALU.mult,
                )
                nc.vector.tensor_reduce(
                    out=accum[:, ds(12 * g, GB)], in_=pr, axis=AX.X, op=ALU.add
                )
                sd = scrp.tile([128, GB, 128], BF16, tag="sd")
                nc.scalar.activation(
                    out=sd, in_=jall[:, :, 0:128], func=ACTF.Square
                )
                nc.vector.tensor_reduce(
                    out=accum[:, ds(12 * g + 4, GB)], in_=sd, axis=AX.X,
                    op=ALU.add,
                )
                sc2 = scrp.tile([128, GB, 128], BF16, tag="sc2")
                nc.gpsimd.tensor_tensor(
                    out=sc2, in0=jall[:, :, 128:256],
                    in1=jall[:, :, 128:256], op=ALU.mult,
                )
                nc.vector.tensor_reduce(
                    out=accum[:, ds(12 * g + 8, GB)], in_=sc2, axis=AX.X,
                    op=ALU.add,
                )

            # ---------- final reduction ----------
            rps = psJ.tile([1, 128], FP32, tag="jd")
            nc.tensor.matmul(
                rps, lhsT=ones1, rhs=accum, start=True, stop=True,
                skip_group_check=True,
            )
            row = constp.tile([1, 128], FP32)
            nc.scalar.copy(row, rps)
            rw = row[:, 0:96].rearrange("o (g t f) -> o g t f", t=3, f=GB)
            f1 = constp.tile([1, 32], FP32)
            f1v = f1.rearrange("o (g f) -> o g f", f=GB)
            nc.vector.tensor_tensor(
                out=f1v, in0=rw[:, :, 1, :], in1=rw[:, :, 2, :], op=ALU.mult
            )
            nc.scalar.activation(out=f1, in_=f1, func=ACTF.Sqrt)
            nc.vector.tensor_scalar_add(f1, f1, EPS)
            f2 = constp.tile([1, 32], FP32)
            f2v = f2.rearrange("o (g f) -> o g f", f=GB)
            nc.vector.reciprocal(f2, f1)
            nc.vector.tensor_tensor(
                out=f2v, in0=rw[:, :, 0, :], in1=f2v, op=ALU.mult
            )
            csum = constp.tile([1, 1], FP32)
            nc.vector.tensor_reduce(out=csum, in_=f2, axis=AX.X, op=ALU.add)
            msum = constp.tile([1, 1], FP32)
            nc.vector.tensor_reduce(
                out=msum, in_=row[:, 96:128], axis=AX.X, op=ALU.add
            )
            part = constp.tile([1, 2], FP32)
            nc.vector.tensor_scalar(
                out=part[:, 0:1], in0=csum, scalar1=-1.0, scalar2=float(B),
                op0=ALU.mult, op1=ALU.add,
            )
            nc.vector.tensor_scalar_mul(part[:, 1:2], msum, 1.0 / QD)
            nc.sync.dma_start(out=out_d, in_=part)

    return nc


_NC_CACHE = {}


def _get_nc():
    if "nc" not in _NC_CACHE:
        _install_drain_fix()
        nc = build_nc()
        _split_waits(nc)
        _NC_CACHE["nc"] = nc
    return _NC_CACHE["nc"]


def _split_waits(nc):
    """This walrus accepts only one sync-wait per instruction; move extras
    onto same-engine NoOps inserted just before."""
    from concourse import mybir
    from bass_rust import SyncInfo

    for f in nc.m.functions:
        for blk in f.blocks:
            insts = list(blk.instructions)
            out = []
            for inst in insts:
                si = inst.sync_info
                waits = list(si.on_wait) if si is not None else []
                if len(waits) > 1:
                    for wi, w in enumerate(waits[:-1]):
                        nop = mybir.InstNoOp(name=f"{inst.name}-wsplit{wi}")
                        nop.engine = inst.engine
                        nop.sync_info = SyncInfo(on_wait=[w], on_update=[])
                        out.append(nop)
                    inst.sync_info = SyncInfo(
                        on_wait=[waits[-1]], on_update=list(si.on_update)
                    )
                out.append(inst)
            blk.instructions = out


N_CORES = 8


def _in_maps(queries, keys, values, k_cg, v_cg):
    consts = host_consts()
    nb = queries.shape[0]
    sh = nb // N_CORES
    return [
        {
            "queries": queries[i * sh : (i + 1) * sh],
            "keys": keys[i * sh : (i + 1) * sh],
            "values": values[i * sh : (i + 1) * sh],
            "k_cg": k_cg[i * sh : (i + 1) * sh],
            "v_cg": v_cg[i * sh : (i + 1) * sh],
            **consts,
        }
        for i in range(N_CORES)
    ]


def kernel(queries, keys, values, k_cg, v_cg):
    from concourse.bass_utils import run_bass_kernel_spmd

    queries = np.ascontiguousarray(np.asarray(queries, dtype=np.float32))
    keys = np.ascontiguousarray(np.asarray(keys, dtype=np.float32))
    values = np.ascontiguousarray(np.asarray(values, dtype=np.float32))
    k_cg = np.ascontiguousarray(np.asarray(k_cg, dtype=np.float32))
    v_cg = np.ascontiguousarray(np.asarray(v_cg, dtype=np.float32))

    nb = queries.shape[0]
    in_maps = _in_maps(queries, keys, values, k_cg, v_cg)
    nc = _get_nc()
    res = run_bass_kernel_spmd(nc, in_maps, core_ids=list(range(N_CORES)))
    total = 0.0
    for i in range(N_CORES):
        part = res.results[i]["out"]
        total += float(part[0, 0]) + float(part[0, 1])
    return np.float32(total / nb)


# revision 24
# speedup vs baseline: 1.2894x; 1.0841x over previous
"""Trainium2 Bass kernel for nn_ForceMatchingLoss (batch-data-parallel over 8 NeuronCores).

Full inputs (B=256) are sharded along the batch dimension: core i computes
batches [32*i, 32*i+32) and returns [sum_b fd_b, sum_b cons_b]; the host
sums the 8 partials and divides by 256 (the loss is a batch mean, so the
"all-reduce" is a trivial host-side sum of 8 scalars).

v3: fp8 DMA-cast loads (SWDGE casts fp32->fp8 in flight, no on-chip cast
ops), out/kbar computed in the d-major direction with fp8 DoubleRow
matmuls (M=128 instead of M=16: 4096 -> ~400 PE cycles/group) then
transposed back in two batched PE transposes, coarse-grained chain in a
32-padded partition layout (kills the 4-per-group partition-restage DMAs
and runs the cg jacobian in fp8), and per-batch jacobian reductions fused
into single accum ops reading PSUM directly.
"""

import numpy as np


# ---------------------------------------------------------------------------
# Workaround for this walrus build: CTRL-type instructions (Drain) only accept
# a single sync-wait; TileContext's tail drain aggregates one wait per logical
# processor.  Split the waits across a chain of drains.
# ---------------------------------------------------------------------------
def _install_drain_fix():
    import concourse.tile as tile
    from bass_rust import ScopedClock, SyncInfo

    if getattr(tile.TileContext, "_drain_fix_installed", False):
        return

    def _drain_and_barrier(self, tick_clock, wait_clock):
        drain_inst = self.nc.sync.drain()
        wait_clock.add_sem_waits(
            drain_inst.ins, ScopedClock({None: tick_clock.global_clock})
        )
        si = drain_inst.ins.sync_info
        waits = list(si.on_wait) if si is not None else []
        if len(waits) > 1:
            drain_inst.ins.sync_info = SyncInfo(
                on_wait=waits[:1], on_update=list(si.on_update)
            )
            for i in range(1, len(waits)):
                d = self.nc.sync.drain()
                d.ins.sync_info = SyncInfo(on_wait=waits[i : i + 1], on_update=[])

        self.nc.all_engine_barrier()
        popped = self.nc._tile_sem_poison_stack.pop()
        assert popped is self._sem_poison
        self.nc.clear_and_free_semaphores(list(self.sems.allocated().values()))
        self.nc.all_engine_barrier()

    tile.TileContext._drain_and_barrier = _drain_and_barrier
    tile.TileContext._drain_fix_installed = True


import concourse.bass as bass
import concourse.tile as tile
from concourse import mybir
from concourse.bass import ds, ts
from concourse.masks import make_identity

FP32 = mybir.dt.float32
BF16 = mybir.dt.bfloat16
F32R = mybir.dt.float32r
F8 = mybir.dt.float8e4
DR = mybir.MatmulPerfMode.DoubleRow
AX = mybir.AxisListType
ALU = mybir.AluOpType
ACTF = mybir.ActivationFunctionType

B = 32          # batches per core
Q = 16
S = 512
M = 8
D = 128
NCH = 4         # s chunks of 128
GB = 4          # batches per group (32-row bands in the scores psum tile)
NG = B // GB    # 8 groups
SCALE = float(D) ** -0.5
EPS = 1e-8
QD = float(Q * D)
PREFETCH = 3    # groups of k/v loads in flight ahead of compute


def r(ap):
    return ap.bitcast(F32R)


def host_consts():
    """Constant mask tensors supplied as extra kernel inputs."""
    # cmask [128, 32]: block-diagonal valid mask for the batched cg scores.
    # row = 32*j + qi (qi valid when < 16), col = 8*j' + m; 1 iff j==j', qi<16.
    cm = np.zeros((128, GB * M), dtype=np.float32)
    for j in range(GB):
        cm[32 * j : 32 * j + Q, M * j : M * j + M] = 1.0
    # qmask1 / qmaskS [128,1]: per-partition valid-row masks for the okb/okcg
    # psum->sbuf copies (junk rows zeroed; kbar half also folds -SCALE).
    q1 = np.zeros((128, 1), dtype=np.float32)
    qs = np.zeros((128, 1), dtype=np.float32)
    for j in range(GB):
        q1[32 * j : 32 * j + Q] = 1.0
        qs[32 * j : 32 * j + Q] = -SCALE
    return {"cmask": cm, "qmask1": q1, "qmaskS": qs}


def build_nc():
    nc = bass.Bass("TRN2", target_bir_lowering=False, debug=False)
    q_d = nc.dram_tensor("queries", [B, Q, D], FP32, kind="ExternalInput").ap()
    k_d = nc.dram_tensor("keys", [B, S, D], FP32, kind="ExternalInput").ap()
    v_d = nc.dram_tensor("values", [B, S, D], FP32, kind="ExternalInput").ap()
    kcg_d = nc.dram_tensor("k_cg", [B, M, D], FP32, kind="ExternalInput").ap()
    vcg_d = nc.dram_tensor("v_cg", [B, M, D], FP32, kind="ExternalInput").ap()
    cm_d = nc.dram_tensor("cmask", [128, GB * M], FP32, kind="ExternalInput").ap()
    q1_d = nc.dram_tensor("qmask1", [128, 1], FP32, kind="ExternalInput").ap()
    qs_d = nc.dram_tensor("qmaskS", [128, 1], FP32, kind="ExternalInput").ap()
    out_d = nc.dram_tensor("out", [1, 2], FP32, kind="ExternalOutput").ap()

    with tile.TileContext(nc) as tc:
        with (
            tc.tile_pool(name="const", bufs=1) as constp,
            tc.tile_pool(name="kvb", bufs=1 + PREFETCH) as kvbp,
            tc.tile_pool(name="kt", bufs=5) as ktp,
            tc.tile_pool(name="sm", bufs=3) as smp,
            tc.tile_pool(name="ptp", bufs=3) as ptp,
            tc.tile_pool(name="vcp", bufs=3) as vcp,
            tc.tile_pool(name="small", bufs=4) as smallp,
            tc.tile_pool(name="ok", bufs=3) as okp,
            tc.tile_pool(name="scr", bufs=4) as scrp,
            tc.tile_pool(name="psSC", bufs=2, space="PSUM") as psSC,
            tc.tile_pool(name="psT", bufs=2, space="PSUM") as psT,
            tc.tile_pool(name="psCG", bufs=2, space="PSUM") as psCG,
            tc.tile_pool(name="psJ", bufs=2, space="PSUM") as psJ,
        ):
            # ---------- k/v prefetch (SWDGE DMA-cast fp32 -> fp8) ----------
            kvq = {}

            def load_group(gg):
                kb = kvbp.tile([128, GB, NCH, 128], F8, tag="kb")
                nc.gpsimd.dma_start(
                    out=kb,
                    in_=k_d[ds(GB * gg, GB)].rearrange(
                        "b (p c) d -> p b c d", c=NCH
                    ),
                )
                vb = kvbp.tile([128, GB, NCH, 128], F8, tag="vb")
                nc.gpsimd.dma_start(
                    out=vb,
                    in_=v_d[ds(GB * gg, GB)].rearrange(
                        "b (p c) d -> p b c d", c=NCH
                    ),
                )
                kvq[gg] = (kb, vb)

            for gg in range(min(PREFETCH, NG)):
                load_group(gg)

            # ---------- constants ----------
            ident = constp.tile([128, 128], FP32)
            make_identity(nc, ident)
            identb = constp.tile([128, 128], BF16)
            nc.scalar.copy(identb, ident)
            identf8 = constp.tile([128, 128], F8)
            nc.scalar.copy(identf8, ident)
            ones1 = constp.tile([128, 1], FP32)
            nc.vector.memset(ones1, 1.0)
            # accumulator columns: per group g, [dot 12g:12g+4 | d2 +4 | c2 +8];
            # cons at 96+g
            accum = constp.tile([128, 128], FP32)
            nc.gpsimd.memset(accum, 0.0)
            cmask = constp.tile([128, GB * M], FP32)
            nc.sync.dma_start(out=cmask, in_=cm_d)
            qmask1 = constp.tile([128, 1], FP32)
            nc.sync.dma_start(out=qmask1, in_=q1_d)
            qmaskS = constp.tile([128, 1], FP32)
            nc.sync.dma_start(out=qmaskS, in_=qs_d)
            qmask64 = constp.tile([128, 1], FP32)
            nc.vector.tensor_scalar_mul(qmask64, qmask1, 1.0 / 64.0)

            # scps junk bands are never written by the M=16 score matmuls;
            # they must hold finite data before the full-tile exp (cols
            # 0:256 get overwritten by the okT reuse each group, which keeps
            # them finite thereafter).  Clear every rotating buffer once.
            for _ in range(2):
                sc0 = psSC.tile([128, 512], FP32, tag="scps")
                nc.vector.memset(sc0, 0.0)

            # ---------- prologue: queries ----------
            # dense load [128 = (b2 q), 4 t, 128 d] in ONE DMA, then transpose
            # and spread into the junk-band layout on-chip.
            q_sb = constp.tile([128, 4, 128], FP32)
            nc.sync.dma_start(
                out=q_sb,
                in_=q_d.rearrange("(t b2) q d -> (b2 q) t d", t=4),
            )
            # qJ [128 d, 8 g, 128 = 4b x 32 cols] bf16, softmax scale folded in
            qJ = constp.tile([128, NG, 128], BF16)
            nc.gpsimd.memset(qJ, 0.0)
            for t in range(4):
                qtps = psSC.tile([128, 512], FP32, tag="scps")
                nc.tensor.transpose(qtps[:, 0:128], q_sb[:, t, :], ident)
                # qtps cols = b2*16+q for batches t*8..t*8+8 = groups 2t, 2t+1
                for gg in range(2):
                    g2 = 2 * t + gg
                    nc.scalar.activation(
                        out=qJ[:, g2, :].rearrange("p (j w) -> p j w", w=32)[
                            :, :, 0:Q
                        ],
                        in_=qtps[:, ds(64 * gg, 64)].rearrange(
                            "p (j w) -> p j w", w=Q
                        ),
                        func=ACTF.Copy,
                        scale=SCALE,
                    )

            # ---------- prologue: coarse-grained tensors ----------
            # kcgT [128 d, 2 t, 128 = (8 b2) x (8 m)] bf16 (no scale)
            kcg_sb = constp.tile([128, 2, 128], FP32)
            nc.sync.dma_start(
                out=kcg_sb,
                in_=kcg_d.rearrange("(t b2) m d -> (b2 m) t d", t=2),
            )
            kcgT = constp.tile([128, 2, 128], BF16)
            kcgtps = psSC.tile([128, 512], FP32, tag="scps")
            for t in range(2):
                nc.tensor.transpose(kcgtps[:, ts(t, 128)], kcg_sb[:, t, :], ident)
            nc.scalar.copy(kcgT[:], kcgtps[:, 0:256].rearrange("p (t x) -> p t x", t=2))

            # Padded cg layout: partition row 32*bl + m holds (batch 4g+bl,
            # coarse key m); junk partitions stay zero so the block-diagonal
            # okcg matmul and the per-batch jac1cg slices read clean data.
            cgkvJ32 = constp.tile([128, NG, 2, 128], FP32)
            nc.gpsimd.memset(cgkvJ32, 0.0)
            kcg8 = constp.tile([128, NG, 128], F8)
            for bl in range(GB):
                src_k = kcg_d.rearrange("(g bl) m d -> bl m g d", bl=GB)[ds(bl, 1)]
                src_v = vcg_d.rearrange("(g bl) m d -> bl m g d", bl=GB)[ds(bl, 1)]
                nc.sync.dma_start(
                    out=r(cgkvJ32[ds(32 * bl, M), :, 0, :]), in_=r(src_k)
                )
                nc.sync.dma_start(
                    out=r(cgkvJ32[ds(32 * bl, M), :, 1, :]), in_=r(src_v)
                )
                nc.gpsimd.dma_start(out=kcg8[ds(32 * bl, M), :, :], in_=src_k)

            # persistent padded pcg tile: valid cols 32*bl + m, junk cols
            # zeroed once and never written again.
            pcgn128 = constp.tile([128, 128], FP32)
            nc.gpsimd.memset(pcgn128, 0.0)

            # ---------- main loop over groups of 4 batches ----------
            # Software-pipelined: stage A (kT transposes + scores + cg
            # scores — PE-heavy, depends only on the prefetched kb) runs one
            # group ahead of stage B, so the PE chews on group g+1's
            # transposes while group g's softmax/copy chains run on
            # Act/DVE/Pool.

            def stage_a(g):
                kb, vb = kvq.pop(g)
                kts = []
                for j in range(GB):
                    ktps = psT.tile([128, 1024], F8, tag="ktps")
                    ktv = ktps.rearrange("p (x two) -> p x two", two=2)[:, :, 0]
                    for c in range(NCH):
                        nc.tensor.transpose(
                            ktv[:, ts(c, 128)], kb[:, j, c, :], identf8
                        )
                    kt = ktp.tile([128, NCH, 128], BF16, tag="kt")
                    ktv_r = ktv.rearrange("p (c x) -> p c x", c=NCH)
                    if j % 2 == 0:
                        nc.vector.tensor_copy(kt[:], ktv_r)
                    else:
                        nc.scalar.copy(kt[:], ktv_r)
                    kts.append(kt)

                # scores: batch j -> rows [32j, 32j+16)
                scps = psSC.tile([128, 512], FP32, tag="scps")
                for j in range(GB):
                    nc.tensor.matmul(
                        scps[ds(32 * j, Q), :],
                        lhsT=qJ[:, g, ds(32 * j, Q)],
                        rhs=kts[j].rearrange("p c x -> p (c x)"),
                        start=True,
                        stop=True,
                        tile_position=(0, 32 * j),
                        skip_group_check=True,
                    )

                # cg scores into the merged cg psum tile:
                # cols [0:64 cgs | 64:320 okcgps | 320:448 pcgps].
                cgall = psCG.tile([128, 512], FP32, tag="cgall")
                nc.tensor.matmul(
                    cgall[:, 0 : GB * M],
                    lhsT=qJ[:, g, :],
                    rhs=kcgT[:, g // 4, ds(8 * ((GB * g) % 16), GB * M)],
                    start=True,
                    stop=True,
                    skip_group_check=True,
                )
                return kb, vb, scps, cgall

            staged = {0: stage_a(0)}

            for g in range(NG):
                if g + PREFETCH < NG:
                    load_group(g + PREFETCH)
                if g + 1 < NG:
                    staged[g + 1] = stage_a(g + 1)
                kb, vb, scps, cgall = staged.pop(g)
                cgs = cgall[:, 0:64]
                okcgps = cgall[:, 64:320]
                pcgps = cgall[:, 320:448]

                # softmax (no max subtraction: |scores| <= ~7)
                ptil = smp.tile([128, 512], BF16, tag="ptil")
                z = smallp.tile([128, 1], FP32, tag="z")
                nc.scalar.activation(out=ptil, in_=scps, func=ACTF.Exp, accum_out=z)
                zr = smallp.tile([128, 1], FP32, tag="zr")
                nc.vector.reciprocal(zr, z)
                ptn = smp.tile([128, 512], F8, tag="ptn")
                nc.vector.tensor_scalar(
                    out=ptn, in0=ptil, scalar1=zr, scalar2=64.0,
                    op0=ALU.mult, op1=ALU.mult,
                )

                # cg softmax chain (Act/DVE small ops)
                ecg = scrp.tile([128, GB * M], FP32, tag="ecg")
                nc.scalar.activation(out=ecg, in_=cgs[:, 0 : GB * M], func=ACTF.Exp)
                em = scrp.tile([128, GB * M], FP32, tag="em")
                zcg = smallp.tile([128, 1], FP32, tag="zcg")
                nc.vector.scalar_tensor_tensor(
                    out=em, in0=ecg, scalar=1.0, in1=cmask,
                    op0=ALU.mult, op1=ALU.mult, accum_out=zcg,
                )
                nc.vector.tensor_scalar_add(zcg, zcg, 1e-20)
                zcgr = smallp.tile([128, 1], FP32, tag="zcgr")
                nc.vector.reciprocal(zcgr, zcg)
                nc.scalar.activation(
                    out=pcgn128.rearrange("p (bl w) -> p bl w", w=32)[:, :, 0:M],
                    in_=em.rearrange("p (bl m) -> p bl m", m=M),
                    func=ACTF.Copy,
                    scale=zcgr,
                )

                # pT via fp8 PE transpose
                ptps = psT.tile([128, 1024], F8, tag="ktps")
                ptv = ptps.rearrange("p (x two) -> p x two", two=2)[:, :, 0]
                for c in range(NCH):
                    nc.tensor.transpose(
                        ptv[:, ts(c, 128)], ptn[:, ts(c, 128)], identf8
                    )
                pT = ptp.tile([128, NCH, 128], F8, tag="pT")
                nc.vector.tensor_copy(pT[:], ptv.rearrange("p (c x) -> p c x", c=NCH))

                # outT/kbarT: fp8 DR, M=128 (d on partitions), 16 tiny
                # matmuls.  The score psum tile is dead after exp, so its
                # cols [0:128] host the DR outputs and [128:256] (bitcast to
                # bf16) host the transposed-back okKO.
                okTps = scps[:, 0:128].rearrange(
                    "p (h j q) -> p h j q", h=2, j=GB
                )
                for j in range(GB):
                    for h, src in ((0, kb), (1, vb)):
                        for cp in range(2):
                            nc.tensor.matmul(
                                okTps[:, h, j, :],
                                lhsT=src[:, j, ds(2 * cp, 2), :],
                                rhs=pT[:, ds(2 * cp, 2), ds(32 * j, Q)],
                                start=(cp == 0),
                                stop=(cp == 1),
                                perf_mode=DR,
                                skip_group_check=True,
                            )

                # c = sum_q p (valid q rows only); SCALE folded into vc below
                c_t = smallp.tile([128, NCH, GB], FP32, tag="c_t")
                nc.vector.tensor_reduce(
                    out=c_t,
                    in_=pT.rearrange("p c (j w) -> p c j w", j=GB)[:, :, :, 0:Q],
                    axis=AX.X,
                    op=ALU.add,
                )
                nc.gpsimd.tensor_scalar_mul(c_t, c_t, SCALE)
                vcs = []
                for j in range(GB):
                    vc = vcp.tile([128, NCH, 128], F8, tag="vc")
                    veng = nc.vector if j % 2 == 0 else nc.gpsimd
                    veng.tensor_tensor(
                        out=vc,
                        in0=vb[:, j, :, :],
                        in1=c_t[:, :, ds(j, 1)].broadcast_to([128, NCH, 128]),
                        op=ALU.mult,
                    )
                    vcs.append(vc)

                # transpose outT/kbarT back to the q-banded layout: staging
                # copy (junk cols zero) then one batched transpose per half.
                okTs = okp.tile([128, 2, GB, 32], BF16, tag="okTs")
                nc.gpsimd.memset(okTs, 0.0)
                nc.vector.tensor_copy(okTs[:, :, :, 0:Q], okTps)
                okKO = scps[:, 128:256].bitcast(BF16).rearrange(
                    "p (h x) -> p h x", h=2
                )
                nc.tensor.transpose(okKO[:, 0, :], okTs[:, 0, :, :], identb)
                nc.tensor.transpose(okKO[:, 1, :], okTs[:, 1, :, :], identb)
                # okb [128, 256] bf16 = [-64*s*kbar | out], junk rows zeroed
                okb = okp.tile([128, 256], BF16, tag="okb")
                nc.scalar.activation(
                    out=okb[:, 0:128], in_=okKO[:, 0, :], func=ACTF.Copy,
                    scale=qmaskS,
                )
                nc.scalar.activation(
                    out=okb[:, 128:256], in_=okKO[:, 1, :], func=ACTF.Copy,
                    scale=qmask64,
                )

                # ---- coarse-grained out/kbar (padded block-diagonal) ----
                nc.tensor.transpose(pcgps, pcgn128, ident)
                pcgTs = scrp.tile([128, 128], FP32, tag="pcgTs")
                nc.scalar.copy(r(pcgTs[:]), pcgps)
                ccg32 = smallp.tile([128, 1], FP32, tag="ccg")
                nc.vector.tensor_reduce(out=ccg32, in_=pcgTs, axis=AX.X, op=ALU.add)
                vccg32 = scrp.tile([128, 128], F8, tag="vccg")
                nc.vector.tensor_scalar(
                    out=vccg32,
                    in0=cgkvJ32[:, g, 1, :],
                    scalar1=ccg32,
                    scalar2=SCALE,
                    op0=ALU.mult,
                    op1=ALU.mult,
                )
                nc.tensor.matmul(
                    okcgps,
                    lhsT=r(pcgTs),
                    rhs=r(cgkvJ32[:, g, :, :].rearrange("p a x -> p (a x)")),
                    start=True,
                    stop=True,
                    skip_group_check=True,
                )
                okcg = okp.tile([128, 256], BF16, tag="okcg")
                nc.scalar.activation(
                    out=okcg[:, 0:128], in_=okcgps[:, 0:128], func=ACTF.Copy,
                    scale=qmaskS,
                )
                nc.scalar.activation(
                    out=okcg[:, 128:256], in_=okcgps[:, 128:256], func=ACTF.Copy,
                    scale=qmask1,
                )

                # consistency for the whole group (junk rows are zero)
                dif = scrp.tile([128, 128], BF16, tag="dif")
                nc.gpsimd.tensor_sub(dif, okb[:, 128:256], okcg[:, 128:256])
                scc = scrp.tile([128, 128], BF16, tag="scc")
                nc.vector.scalar_tensor_tensor(
                    out=scc, in0=dif, scalar=1.0, in1=dif,
                    op0=ALU.mult, op1=ALU.mult,
                    accum_out=accum[:, ds(96 + g, 1)],
                )

                # ---- per-batch jacobians (2 per psum tile) ----
                jall = scrp.tile([128, GB, 256], BF16, tag="jall")
                for pair in range(2):
                    jp = psJ.tile([128, 2, 256], FP32, tag="jd")
                    for jj in range(2):
                        j = 2 * pair + jj
                        # dense jac (x64): fp8 DR chunk-pairs + out^T(-64 s kbar)
                        for cp in range(2):
                            nc.tensor.matmul(
                                jp[:, jj, 0:128],
                                lhsT=vcs[j][:, ds(2 * cp, 2), :],
                                rhs=kb[:, j, ds(2 * cp, 2), :],
                                start=(cp == 0),
                                stop=False,
                                perf_mode=DR,
                                skip_group_check=True,
                            )
                        nc.tensor.matmul(
                            jp[:, jj, 0:128],
                            lhsT=okb[ds(32 * j, Q), 128:256],
                            rhs=okb[ds(32 * j, Q), 0:128],
                            start=False,
                            stop=True,
                            tile_position=(32 * j, 0),
                            skip_group_check=True,
                        )
                        # cg jac: fp8 jac1 + bf16 jac2, same psum region
                        nc.tensor.matmul(
                            jp[:, jj, 128:256],
                            lhsT=vccg32[ds(32 * j, M), :],
                            rhs=kcg8[ds(32 * j, M), g, :],
                            start=True,
                            stop=False,
                            tile_position=(32 * j, 0),
                            skip_group_check=True,
                        )
                        nc.tensor.matmul(
                            jp[:, jj, 128:256],
                            lhsT=okcg[ds(32 * j, Q), 128:256],
                            rhs=okcg[ds(32 * j, Q), 0:128],
                            start=False,
                            stop=True,
                            tile_position=(32 * j, 0),
                            skip_group_check=True,
                        )
                    # bounce the pair to SBUF bf16 for the reductions
                    nc.scalar.copy(jall[:, ds(2 * pair, 2), :], jp)

                # group-level dot/norm reductions off SBUF
                pr = scrp.tile([128, GB, 128], BF16, tag="pr")
                nc.vector.tensor_tensor(
                    out=pr, in0=jall[:, :, 0:128], in1=jall[:, :, 128:256],
                    op=ALU.mult,
                )
                nc.vector.tensor_reduce(
                    out=accum[:, ds(12 * g, GB)], in_=pr, axis=AX.X, op=ALU.add
                )
                sd = scrp.tile([128, GB, 128], BF16, tag="sd")
                nc.scalar.activation(
                    out=sd, in_=jall[:, :, 0:128], func=ACTF.Square
                )
                nc.vector.tensor_reduce(
                    out=accum[:, ds(12 * g + 4, GB)], in_=sd, axis=AX.X,
                    op=ALU.add,
                )
                sc2 = scrp.tile([128, GB, 128], BF16, tag="sc2")
                nc.gpsimd.tensor_tensor(
                    out=sc2, in0=jall[:, :, 128:256],
                    in1=jall[:, :, 128:256], op=ALU.mult,
                )
                nc.vector.tensor_reduce(
                    out=accum[:, ds(12 * g + 8, GB)], in_=sc2, axis=AX.X,
                    op=ALU.add,
                )

            # ---------- final reduction ----------
            rps = psJ.tile([1, 128], FP32, tag="jd")
            nc.tensor.matmul(
                rps, lhsT=ones1, rhs=accum, start=True, stop=True,
                skip_group_check=True,
            )
            row = constp.tile([1, 128], FP32)
            nc.scalar.copy(row, rps)
            rw = row[:, 0:96].rearrange("o (g t f) -> o g t f", t=3, f=GB)
            f1 = constp.tile([1, 32], FP32)
            f1v = f1.rearrange("o (g f) -> o g f", f=GB)
            nc.vector.tensor_tensor(
                out=f1v, in0=rw[:, :, 1, :], in1=rw[:, :, 2, :], op=ALU.mult
            )
            nc.scalar.activation(out=f1, in_=f1, func=ACTF.Sqrt)
            nc.vector.tensor_scalar_add(f1, f1, EPS)
            f2 = constp.tile([1, 32], FP32)
            f2v = f2.rearrange("o (g f) -> o g f", f=GB)
            nc.vector.reciprocal(f2, f1)
            nc.vector.tensor_tensor(
                out=f2v, in0=rw[:, :, 0, :], in1=f2v, op=ALU.mult
            )
            csum = constp.tile([1, 1], FP32)
            nc.vector.tensor_reduce(out=csum, in_=f2, axis=AX.X, op=ALU.add)
            msum = constp.tile([1, 1], FP32)
            nc.vector.tensor_reduce(
                out=msum, in_=row[:, 96:128], axis=AX.X, op=ALU.add
            )
            part = constp.tile([1, 2], FP32)
            nc.vector.tensor_scalar(
                out=part[:, 0:1], in0=csum, scalar1=-1.0, scalar2=float(B),
                op0=ALU.mult, op1=ALU.add,
            )
            nc.vector.tensor_scalar_mul(part[:, 1:2], msum, 1.0 / QD)
            nc.sync.dma_start(out=out_d, in_=part)

    return nc


_NC_CACHE = {}


def _get_nc():
    if "nc" not in _NC_CACHE:
        _install_drain_fix()
        nc = build_nc()
        _split_waits(nc)
        _NC_CACHE["nc"] = nc
    return _NC_CACHE["nc"]


def _split_waits(nc):
    """This walrus accepts only one sync-wait per instruction; move extras
    onto same-engine NoOps inserted just before."""
    from concourse import mybir
    from bass_rust import SyncInfo

    for f in nc.m.functions:
        for blk in f.blocks:
            insts = list(blk.instructions)
            out = []
            for inst in insts:
                si = inst.sync_info
                waits = list(si.on_wait) if si is not None else []
                if len(waits) > 1:
                    for wi, w in enumerate(waits[:-1]):
                        nop = mybir.InstNoOp(name=f"{inst.name}-wsplit{wi}")
                        nop.engine = inst.engine
                        nop.sync_info = SyncInfo(on_wait=[w], on_update=[])
                        out.append(nop)
                    inst.sync_info = SyncInfo(
                        on_wait=[waits[-1]], on_update=list(si.on_update)
                    )
                out.append(inst)
            blk.instructions = out


N_CORES = 8


def _in_maps(queries, keys, values, k_cg, v_cg):
    consts = host_consts()
    nb = queries.shape[0]
    sh = nb // N_CORES
    return [
        {
            "queries": queries[i * sh : (i + 1) * sh],
            "keys": keys[i * sh : (i + 1) * sh],
            "values": values[i * sh : (i + 1) * sh],
            "k_cg": k_cg[i * sh : (i + 1) * sh],
            "v_cg": v_cg[i * sh : (i + 1) * sh],
            **consts,
        }
        for i in range(N_CORES)
    ]


def kernel(queries, keys, values, k_cg, v_cg):
    from concourse.bass_utils import run_bass_kernel_spmd

    queries = np.ascontiguousarray(np.asarray(queries, dtype=np.float32))
    keys = np.ascontiguousarray(np.asarray(keys, dtype=np.float32))
    values = np.ascontiguousarray(np.asarray(values, dtype=np.float32))
    k_cg = np.ascontiguousarray(np.asarray(k_cg, dtype=np.float32))
    v_cg = np.ascontiguousarray(np.asarray(v_cg, dtype=np.float32))

    nb = queries.shape[0]
    in_maps = _in_maps(queries, keys, values, k_cg, v_cg)
    nc = _get_nc()
    res = run_bass_kernel_spmd(nc, in_maps, core_ids=list(range(N_CORES)))
    total = 0.0
    for i in range(N_CORES):
        part = res.results[i]["out"]
        total += float(part[0, 0]) + float(part[0, 1])
    return np.float32(total / nb)


# revision 27
# speedup vs baseline: 1.3645x; 1.0583x over previous
"""Trainium2 Bass kernel for nn_ForceMatchingLoss (batch-data-parallel over 8 NeuronCores).

Full inputs (B=256) are sharded along the batch dimension: core i computes
batches [32*i, 32*i+32) and returns [sum_b fd_b, sum_b cons_b]; the host
sums the 8 partials and divides by 256 (the loss is a batch mean, so the
"all-reduce" is a trivial host-side sum of 8 scalars).

v3: fp8 DMA-cast loads (SWDGE casts fp32->fp8 in flight, no on-chip cast
ops), out/kbar computed in the d-major direction with fp8 DoubleRow
matmuls (M=128 instead of M=16: 4096 -> ~400 PE cycles/group) then
transposed back in two batched PE transposes, coarse-grained chain in a
32-padded partition layout (kills the 4-per-group partition-restage DMAs
and runs the cg jacobian in fp8), and per-batch jacobian reductions fused
into single accum ops reading PSUM directly.
"""

import numpy as np


# ---------------------------------------------------------------------------
# Workaround for this walrus build: CTRL-type instructions (Drain) only accept
# a single sync-wait; TileContext's tail drain aggregates one wait per logical
# processor.  Split the waits across a chain of drains.
# ---------------------------------------------------------------------------
def _install_drain_fix():
    import concourse.tile as tile
    from bass_rust import ScopedClock, SyncInfo

    if getattr(tile.TileContext, "_drain_fix_installed", False):
        return

    def _drain_and_barrier(self, tick_clock, wait_clock):
        drain_inst = self.nc.sync.drain()
        wait_clock.add_sem_waits(
            drain_inst.ins, ScopedClock({None: tick_clock.global_clock})
        )
        si = drain_inst.ins.sync_info
        waits = list(si.on_wait) if si is not None else []
        if len(waits) > 1:
            drain_inst.ins.sync_info = SyncInfo(
                on_wait=waits[:1], on_update=list(si.on_update)
            )
            for i in range(1, len(waits)):
                d = self.nc.sync.drain()
                d.ins.sync_info = SyncInfo(on_wait=waits[i : i + 1], on_update=[])

        self.nc.all_engine_barrier()
        popped = self.nc._tile_sem_poison_stack.pop()
        assert popped is self._sem_poison
        self.nc.clear_and_free_semaphores(list(self.sems.allocated().values()))
        self.nc.all_engine_barrier()

    tile.TileContext._drain_and_barrier = _drain_and_barrier
    tile.TileContext._drain_fix_installed = True


import concourse.bass as bass
import concourse.tile as tile
from concourse import mybir
from concourse.bass import ds, ts
from concourse.masks import make_identity

FP32 = mybir.dt.float32
BF16 = mybir.dt.bfloat16
F32R = mybir.dt.float32r
F8 = mybir.dt.float8e4
DR = mybir.MatmulPerfMode.DoubleRow
AX = mybir.AxisListType
ALU = mybir.AluOpType
ACTF = mybir.ActivationFunctionType

B = 32          # batches per core
Q = 16
S = 512
M = 8
D = 128
NCH = 4         # s chunks of 128
GB = 4          # batches per group (32-row bands in the scores psum tile)
NG = B // GB    # 8 groups
SCALE = float(D) ** -0.5
EPS = 1e-8
QD = float(Q * D)
PREFETCH = 3    # groups of k/v loads in flight ahead of compute


def r(ap):
    return ap.bitcast(F32R)


def host_consts():
    """Constant mask tensors supplied as extra kernel inputs."""
    # cmask [128, 32]: block-diagonal valid mask for the batched cg scores.
    # row = 32*j + qi (qi valid when < 16), col = 8*j' + m; 1 iff j==j', qi<16.
    cm = np.zeros((128, GB * M), dtype=np.float32)
    for j in range(GB):
        cm[32 * j : 32 * j + Q, M * j : M * j + M] = 1.0
    # qmask1 / qmaskS [128,1]: per-partition valid-row masks for the okb/okcg
    # psum->sbuf copies (junk rows zeroed; kbar half also folds -SCALE).
    q1 = np.zeros((128, 1), dtype=np.float32)
    qs = np.zeros((128, 1), dtype=np.float32)
    for j in range(GB):
        q1[32 * j : 32 * j + Q] = 1.0
        qs[32 * j : 32 * j + Q] = -SCALE
    return {"cmask": cm, "qmask1": q1, "qmaskS": qs}


def build_nc():
    nc = bass.Bass("TRN2", target_bir_lowering=False, debug=False)
    q_d = nc.dram_tensor("queries", [B, Q, D], FP32, kind="ExternalInput").ap()
    k_d = nc.dram_tensor("keys", [B, S, D], FP32, kind="ExternalInput").ap()
    v_d = nc.dram_tensor("values", [B, S, D], FP32, kind="ExternalInput").ap()
    kcg_d = nc.dram_tensor("k_cg", [B, M, D], FP32, kind="ExternalInput").ap()
    vcg_d = nc.dram_tensor("v_cg", [B, M, D], FP32, kind="ExternalInput").ap()
    cm_d = nc.dram_tensor("cmask", [128, GB * M], FP32, kind="ExternalInput").ap()
    q1_d = nc.dram_tensor("qmask1", [128, 1], FP32, kind="ExternalInput").ap()
    qs_d = nc.dram_tensor("qmaskS", [128, 1], FP32, kind="ExternalInput").ap()
    out_d = nc.dram_tensor("out", [1, 2], FP32, kind="ExternalOutput").ap()

    with tile.TileContext(nc) as tc:
        with (
            tc.tile_pool(name="const", bufs=1) as constp,
            tc.tile_pool(name="kvb", bufs=1 + PREFETCH) as kvbp,
            tc.tile_pool(name="kt", bufs=5) as ktp,
            tc.tile_pool(name="sm", bufs=3) as smp,
            tc.tile_pool(name="ptp", bufs=3) as ptp,
            tc.tile_pool(name="vcp", bufs=3) as vcp,
            tc.tile_pool(name="small", bufs=4) as smallp,
            tc.tile_pool(name="ok", bufs=3) as okp,
            tc.tile_pool(name="scr", bufs=4) as scrp,
            tc.tile_pool(name="psSC", bufs=2, space="PSUM") as psSC,
            tc.tile_pool(name="psT", bufs=2, space="PSUM") as psT,
            tc.tile_pool(name="psCG", bufs=2, space="PSUM") as psCG,
            tc.tile_pool(name="psJ", bufs=2, space="PSUM") as psJ,
        ):
            # ---------- k/v prefetch (SWDGE DMA-cast fp32 -> fp8) ----------
            kvq = {}

            def load_group(gg):
                kb = kvbp.tile([128, GB, NCH, 128], F8, tag="kb")
                nc.gpsimd.dma_start(
                    out=kb,
                    in_=k_d[ds(GB * gg, GB)].rearrange(
                        "b (p c) d -> p b c d", c=NCH
                    ),
                )
                vb = kvbp.tile([128, GB, NCH, 128], F8, tag="vb")
                nc.gpsimd.dma_start(
                    out=vb,
                    in_=v_d[ds(GB * gg, GB)].rearrange(
                        "b (p c) d -> p b c d", c=NCH
                    ),
                )
                kvq[gg] = (kb, vb)

            for gg in range(min(PREFETCH, NG)):
                load_group(gg)

            # ---------- constants ----------
            ident = constp.tile([128, 128], FP32)
            make_identity(nc, ident)
            identb = constp.tile([128, 128], BF16)
            nc.scalar.copy(identb, ident)
            identf8 = constp.tile([128, 128], F8)
            nc.scalar.copy(identf8, ident)
            ones1 = constp.tile([128, 1], FP32)
            nc.vector.memset(ones1, 1.0)
            # accumulator columns: per group g, [dot 12g:12g+4 | d2 +4 | c2 +8];
            # cons at 96+g
            accum = constp.tile([128, 128], FP32)
            nc.gpsimd.memset(accum, 0.0)
            cmask = constp.tile([128, GB * M], FP32)
            nc.sync.dma_start(out=cmask, in_=cm_d)
            qmask1 = constp.tile([128, 1], FP32)
            nc.sync.dma_start(out=qmask1, in_=q1_d)
            qmaskS = constp.tile([128, 1], FP32)
            nc.sync.dma_start(out=qmaskS, in_=qs_d)
            qmask64 = constp.tile([128, 1], FP32)
            nc.vector.tensor_scalar_mul(qmask64, qmask1, 1.0 / 64.0)

            # scps junk bands are never written by the M=16 score matmuls;
            # they must hold finite data before the full-tile exp (cols
            # 0:256 get overwritten by the okT reuse each group, which keeps
            # them finite thereafter).  Clear every rotating buffer once.
            for _ in range(2):
                sc0 = psSC.tile([128, 512], FP32, tag="scps")
                nc.vector.memset(sc0, 0.0)

            # ---------- prologue: queries ----------
            # dense load [128 = (b2 q), 4 t, 128 d] in ONE DMA, then transpose
            # and spread into the junk-band layout on-chip.
            q_sb = constp.tile([128, 4, 128], FP32)
            nc.sync.dma_start(
                out=q_sb,
                in_=q_d.rearrange("(t b2) q d -> (b2 q) t d", t=4),
            )
            # qJ [128 d, 8 g, 128 = 4b x 32 cols] fp8, softmax scale folded in
            qJ = constp.tile([128, NG, 128], F8)
            nc.gpsimd.memset(qJ, 0.0)
            for t in range(4):
                qtps = psSC.tile([128, 512], FP32, tag="scps")
                nc.tensor.transpose(qtps[:, 0:128], q_sb[:, t, :], ident)
                # qtps cols = b2*16+q for batches t*8..t*8+8 = groups 2t, 2t+1
                for gg in range(2):
                    g2 = 2 * t + gg
                    nc.scalar.activation(
                        out=qJ[:, g2, :].rearrange("p (j w) -> p j w", w=32)[
                            :, :, 0:Q
                        ],
                        in_=qtps[:, ds(64 * gg, 64)].rearrange(
                            "p (j w) -> p j w", w=Q
                        ),
                        func=ACTF.Copy,
                        scale=SCALE,
                    )

            # ---------- prologue: coarse-grained tensors ----------
            # kcgT [128 d, 2 t, 128 = (8 b2) x (8 m)] bf16 (no scale)
            kcg_sb = constp.tile([128, 2, 128], FP32)
            nc.sync.dma_start(
                out=kcg_sb,
                in_=kcg_d.rearrange("(t b2) m d -> (b2 m) t d", t=2),
            )
            kcgT = constp.tile([128, 2, 128], F8)
            kcgtps = psSC.tile([128, 512], FP32, tag="scps")
            for t in range(2):
                nc.tensor.transpose(kcgtps[:, ts(t, 128)], kcg_sb[:, t, :], ident)
            nc.scalar.copy(kcgT[:], kcgtps[:, 0:256].rearrange("p (t x) -> p t x", t=2))

            # Padded cg layout: partition row 32*bl + m holds (batch 4g+bl,
            # coarse key m); junk partitions stay zero so the block-diagonal
            # okcg matmul and the per-batch jac1cg slices read clean data.
            cgkvJ32 = constp.tile([128, NG, 2, 128], FP32)
            nc.gpsimd.memset(cgkvJ32, 0.0)
            kcg8 = constp.tile([128, NG, 128], F8)
            for bl in range(GB):
                src_k = kcg_d.rearrange("(g bl) m d -> bl m g d", bl=GB)[ds(bl, 1)]
                src_v = vcg_d.rearrange("(g bl) m d -> bl m g d", bl=GB)[ds(bl, 1)]
                nc.sync.dma_start(
                    out=r(cgkvJ32[ds(32 * bl, M), :, 0, :]), in_=r(src_k)
                )
                nc.sync.dma_start(
                    out=r(cgkvJ32[ds(32 * bl, M), :, 1, :]), in_=r(src_v)
                )
                nc.gpsimd.dma_start(out=kcg8[ds(32 * bl, M), :, :], in_=src_k)

            # persistent padded pcg tile: valid cols 32*bl + m, junk cols
            # zeroed once and never written again.
            pcgn128 = constp.tile([128, 128], FP32)
            nc.gpsimd.memset(pcgn128, 0.0)

            # ---------- main loop over groups of 4 batches ----------
            # Software-pipelined: stage A (kT transposes + scores + cg
            # scores — PE-heavy, depends only on the prefetched kb) runs one
            # group ahead of stage B, so the PE chews on group g+1's
            # transposes while group g's softmax/copy chains run on
            # Act/DVE/Pool.

            def stage_a(g):
                kb, vb = kvq.pop(g)
                kts = []
                for j in range(GB):
                    ktps = psT.tile([128, 1024], F8, tag="ktps")
                    ktv = ktps.rearrange("p (x two) -> p x two", two=2)[:, :, 0]
                    for c in range(NCH):
                        nc.tensor.transpose(
                            ktv[:, ts(c, 128)], kb[:, j, c, :], identf8
                        )
                    kt = ktp.tile([128, NCH, 128], F8, tag="kt")
                    ktv_r = ktv.rearrange("p (c x) -> p c x", c=NCH)
                    if j % 2 == 0:
                        nc.vector.tensor_copy(kt[:], ktv_r)
                    else:
                        nc.scalar.copy(kt[:], ktv_r)
                    kts.append(kt)

                # scores: batch j -> rows [32j, 32j+16)
                scps = psSC.tile([128, 512], FP32, tag="scps")
                for j in range(GB):
                    nc.tensor.matmul(
                        scps[ds(32 * j, Q), :],
                        lhsT=qJ[:, g, ds(32 * j, Q)],
                        rhs=kts[j].rearrange("p c x -> p (c x)"),
                        start=True,
                        stop=True,
                        perf_mode=mybir.MatmulPerfMode.DoublePixel,
                        tile_position=(0, 32 * j),
                        skip_group_check=True,
                    )

                # cg scores into the merged cg psum tile:
                # cols [0:64 cgs | 64:320 okcgps | 320:448 pcgps].
                cgall = psCG.tile([128, 512], FP32, tag="cgall")
                nc.tensor.matmul(
                    cgall[:, 0 : GB * M],
                    lhsT=qJ[:, g, :],
                    rhs=kcgT[:, g // 4, ds(8 * ((GB * g) % 16), GB * M)],
                    start=True,
                    stop=True,
                    skip_group_check=True,
                )
                return kb, vb, scps, cgall

            staged = {0: stage_a(0)}

            for g in range(NG):
                if g + PREFETCH < NG:
                    load_group(g + PREFETCH)
                if g + 1 < NG:
                    staged[g + 1] = stage_a(g + 1)
                kb, vb, scps, cgall = staged.pop(g)
                cgs = cgall[:, 0:64]
                okcgps = cgall[:, 64:320]
                pcgps = cgall[:, 320:448]

                # softmax (no max subtraction: |scores| <= ~7)
                ptil = smp.tile([128, 512], BF16, tag="ptil")
                z = smallp.tile([128, 1], FP32, tag="z")
                nc.scalar.activation(out=ptil, in_=scps, func=ACTF.Exp, accum_out=z)
                zr = smallp.tile([128, 1], FP32, tag="zr")
                nc.vector.reciprocal(zr, z)
                ptn = smp.tile([128, 512], F8, tag="ptn")
                nc.vector.tensor_scalar(
                    out=ptn, in0=ptil, scalar1=zr, scalar2=64.0,
                    op0=ALU.mult, op1=ALU.mult,
                )

                # cg softmax chain (Act/DVE small ops)
                ecg = scrp.tile([128, GB * M], FP32, tag="ecg")
                nc.scalar.activation(out=ecg, in_=cgs[:, 0 : GB * M], func=ACTF.Exp)
                em = scrp.tile([128, GB * M], FP32, tag="em")
                zcg = smallp.tile([128, 1], FP32, tag="zcg")
                nc.vector.scalar_tensor_tensor(
                    out=em, in0=ecg, scalar=1.0, in1=cmask,
                    op0=ALU.mult, op1=ALU.mult, accum_out=zcg,
                )
                nc.vector.tensor_scalar_add(zcg, zcg, 1e-20)
                zcgr = smallp.tile([128, 1], FP32, tag="zcgr")
                nc.vector.reciprocal(zcgr, zcg)
                nc.scalar.activation(
                    out=pcgn128.rearrange("p (bl w) -> p bl w", w=32)[:, :, 0:M],
                    in_=em.rearrange("p (bl m) -> p bl m", m=M),
                    func=ACTF.Copy,
                    scale=zcgr,
                )

                # pT via fp8 PE transpose
                ptps = psT.tile([128, 1024], F8, tag="ktps")
                ptv = ptps.rearrange("p (x two) -> p x two", two=2)[:, :, 0]
                for c in range(NCH):
                    nc.tensor.transpose(
                        ptv[:, ts(c, 128)], ptn[:, ts(c, 128)], identf8
                    )
                pT = ptp.tile([128, NCH, 128], F8, tag="pT")
                nc.vector.tensor_copy(pT[:], ptv.rearrange("p (c x) -> p c x", c=NCH))

                # outT/kbarT: fp8 DR, M=128 (d on partitions), 16 tiny
                # matmuls.  The score psum tile is dead after exp, so its
                # cols [0:128] host the DR outputs and [128:256] (bitcast to
                # bf16) host the transposed-back okKO.
                okTps = scps[:, 0:128].rearrange(
                    "p (h j q) -> p h j q", h=2, j=GB
                )
                for j in range(GB):
                    for h, src in ((0, kb), (1, vb)):
                        for cp in range(2):
                            nc.tensor.matmul(
                                okTps[:, h, j, :],
                                lhsT=src[:, j, ds(2 * cp, 2), :],
                                rhs=pT[:, ds(2 * cp, 2), ds(32 * j, Q)],
                                start=(cp == 0),
                                stop=(cp == 1),
                                perf_mode=DR,
                                skip_group_check=True,
                            )

                # c = sum_q p (valid q rows only); SCALE folded into vc below
                c_t = smallp.tile([128, NCH, GB], FP32, tag="c_t")
                nc.vector.tensor_reduce(
                    out=c_t,
                    in_=pT.rearrange("p c (j w) -> p c j w", j=GB)[:, :, :, 0:Q],
                    axis=AX.X,
                    op=ALU.add,
                )
                nc.gpsimd.tensor_scalar_mul(c_t, c_t, SCALE)
                vcs = []
                for j in range(GB):
                    vc = vcp.tile([128, NCH, 128], F8, tag="vc")
                    nc.vector.tensor_tensor(
                        out=vc,
                        in0=vb[:, j, :, :],
                        in1=c_t[:, :, ds(j, 1)].broadcast_to([128, NCH, 128]),
                        op=ALU.mult,
                    )
                    vcs.append(vc)

                # transpose outT/kbarT back to the q-banded layout: staging
                # copy (junk cols zero) then one batched transpose per half.
                okTs = okp.tile([128, 2, GB, 32], BF16, tag="okTs")
                nc.gpsimd.memset(okTs, 0.0)
                nc.vector.tensor_copy(okTs[:, :, :, 0:Q], okTps)
                okKO = scps[:, 128:256].bitcast(BF16).rearrange(
                    "p (h x) -> p h x", h=2
                )
                nc.tensor.transpose(okKO[:, 0, :], okTs[:, 0, :, :], identb)
                nc.tensor.transpose(okKO[:, 1, :], okTs[:, 1, :, :], identb)
                # okb [128, 256] bf16 = [-64*s*kbar | out], junk rows zeroed
                okb = okp.tile([128, 256], BF16, tag="okb")
                nc.scalar.activation(
                    out=okb[:, 0:128], in_=okKO[:, 0, :], func=ACTF.Copy,
                    scale=qmaskS,
                )
                nc.scalar.activation(
                    out=okb[:, 128:256], in_=okKO[:, 1, :], func=ACTF.Copy,
                    scale=qmask64,
                )

                # ---- coarse-grained out/kbar (padded block-diagonal) ----
                nc.tensor.transpose(pcgps, pcgn128, ident)
                pcgTs = scrp.tile([128, 128], FP32, tag="pcgTs")
                nc.scalar.copy(r(pcgTs[:]), pcgps)
                ccg32 = smallp.tile([128, 1], FP32, tag="ccg")
                nc.vector.tensor_reduce(out=ccg32, in_=pcgTs, axis=AX.X, op=ALU.add)
                vccg32 = scrp.tile([128, 128], F8, tag="vccg")
                nc.vector.tensor_scalar(
                    out=vccg32,
                    in0=cgkvJ32[:, g, 1, :],
                    scalar1=ccg32,
                    scalar2=SCALE,
                    op0=ALU.mult,
                    op1=ALU.mult,
                )
                nc.tensor.matmul(
                    okcgps,
                    lhsT=r(pcgTs),
                    rhs=r(cgkvJ32[:, g, :, :].rearrange("p a x -> p (a x)")),
                    start=True,
                    stop=True,
                    skip_group_check=True,
                )
                okcg = okp.tile([128, 256], BF16, tag="okcg")
                nc.scalar.activation(
                    out=okcg[:, 0:128], in_=okcgps[:, 0:128], func=ACTF.Copy,
                    scale=qmaskS,
                )
                nc.scalar.activation(
                    out=okcg[:, 128:256], in_=okcgps[:, 128:256], func=ACTF.Copy,
                    scale=qmask1,
                )

                # consistency for the whole group (junk rows are zero)
                dif = scrp.tile([128, 128], BF16, tag="dif")
                nc.vector.tensor_sub(dif, okb[:, 128:256], okcg[:, 128:256])
                scc = scrp.tile([128, 128], BF16, tag="scc")
                nc.vector.scalar_tensor_tensor(
                    out=scc, in0=dif, scalar=1.0, in1=dif,
                    op0=ALU.mult, op1=ALU.mult,
                    accum_out=accum[:, ds(96 + g, 1)],
                )

                # ---- per-batch jacobians (2 per psum tile) ----
                jall = scrp.tile([128, GB, 256], BF16, tag="jall")
                for pair in range(2):
                    jp = psJ.tile([128, 2, 256], FP32, tag="jd")
                    for jj in range(2):
                        j = 2 * pair + jj
                        # dense jac (x64): fp8 DR chunk-pairs + out^T(-64 s kbar)
                        for cp in range(2):
                            nc.tensor.matmul(
                                jp[:, jj, 0:128],
                                lhsT=vcs[j][:, ds(2 * cp, 2), :],
                                rhs=kb[:, j, ds(2 * cp, 2), :],
                                start=(cp == 0),
                                stop=False,
                                perf_mode=DR,
                                skip_group_check=True,
                            )
                        nc.tensor.matmul(
                            jp[:, jj, 0:128],
                            lhsT=okb[ds(32 * j, Q), 128:256],
                            rhs=okb[ds(32 * j, Q), 0:128],
                            start=False,
                            stop=True,
                            tile_position=(32 * j, 0),
                            skip_group_check=True,
                        )
                        # cg jac: fp8 jac1 + bf16 jac2, same psum region
                        nc.tensor.matmul(
                            jp[:, jj, 128:256],
                            lhsT=vccg32[ds(32 * j, M), :],
                            rhs=kcg8[ds(32 * j, M), g, :],
                            start=True,
                            stop=False,
                            tile_position=(32 * j, 0),
                            skip_group_check=True,
                        )
                        nc.tensor.matmul(
                            jp[:, jj, 128:256],
                            lhsT=okcg[ds(32 * j, Q), 128:256],
                            rhs=okcg[ds(32 * j, Q), 0:128],
                            start=False,
                            stop=True,
                            tile_position=(32 * j, 0),
                            skip_group_check=True,
                        )
                    # bounce the pair to SBUF bf16 for the reductions
                    nc.scalar.copy(jall[:, ds(2 * pair, 2), :], jp)

                # group-level dot/norm reductions off SBUF
                pr = scrp.tile([128, GB, 128], BF16, tag="pr")
                nc.vector.tensor_tensor(
                    out=pr, in0=jall[:, :, 0:128], in1=jall[:, :, 128:256],
                    op=ALU.mult,
                )
                nc.vector.tensor_reduce(
                    out=accum[:, ds(12 * g, GB)], in_=pr, axis=AX.X, op=ALU.add
                )
                sq = scrp.tile([128, GB, 256], BF16, tag="sq")
                nc.scalar.activation(out=sq, in_=jall, func=ACTF.Square)
                nc.vector.tensor_reduce(
                    out=accum[:, ds(12 * g + 4, 8)],
                    in_=sq.rearrange("p j (h x) -> p (j h) x", h=2),
                    axis=AX.X,
                    op=ALU.add,
                )

            # ---------- final reduction ----------
            rps = psJ.tile([1, 128], FP32, tag="jd")
            nc.tensor.matmul(
                rps, lhsT=ones1, rhs=accum, start=True, stop=True,
                skip_group_check=True,
            )
            row = constp.tile([1, 128], FP32)
            nc.scalar.copy(row, rps)
            rw = row[:, 0:96].rearrange("o (g x) -> o g x", x=12)
            sqv = rw[:, :, 4:12].rearrange("o g (j h) -> o g j h", h=2)
            f1 = constp.tile([1, 32], FP32)
            f1v = f1.rearrange("o (g f) -> o g f", f=GB)
            nc.vector.tensor_tensor(
                out=f1v, in0=sqv[:, :, :, 0], in1=sqv[:, :, :, 1], op=ALU.mult
            )
            nc.scalar.activation(out=f1, in_=f1, func=ACTF.Sqrt)
            nc.vector.tensor_scalar_add(f1, f1, EPS)
            f2 = constp.tile([1, 32], FP32)
            f2v = f2.rearrange("o (g f) -> o g f", f=GB)
            nc.vector.reciprocal(f2, f1)
            nc.vector.tensor_tensor(
                out=f2v, in0=rw[:, :, 0:4], in1=f2v, op=ALU.mult
            )
            csum = constp.tile([1, 1], FP32)
            nc.vector.tensor_reduce(out=csum, in_=f2, axis=AX.X, op=ALU.add)
            msum = constp.tile([1, 1], FP32)
            nc.vector.tensor_reduce(
                out=msum, in_=row[:, 96:128], axis=AX.X, op=ALU.add
            )
            part = constp.tile([1, 2], FP32)
            nc.vector.tensor_scalar(
                out=part[:, 0:1], in0=csum, scalar1=-1.0, scalar2=float(B),
                op0=ALU.mult, op1=ALU.add,
            )
            nc.vector.tensor_scalar_mul(part[:, 1:2], msum, 1.0 / QD)
            nc.sync.dma_start(out=out_d, in_=part)

    return nc


_NC_CACHE = {}


def _get_nc():
    if "nc" not in _NC_CACHE:
        _install_drain_fix()
        nc = build_nc()
        _split_waits(nc)
        _NC_CACHE["nc"] = nc
    return _NC_CACHE["nc"]


def _split_waits(nc):
    """This walrus accepts only one sync-wait per instruction; move extras
    onto same-engine NoOps inserted just before."""
    from concourse import mybir
    from bass_rust import SyncInfo

    for f in nc.m.functions:
        for blk in f.blocks:
            insts = list(blk.instructions)
            out = []
            for inst in insts:
                si = inst.sync_info
                waits = list(si.on_wait) if si is not None else []
                if len(waits) > 1:
                    for wi, w in enumerate(waits[:-1]):
                        nop = mybir.InstNoOp(name=f"{inst.name}-wsplit{wi}")
                        nop.engine = inst.engine
                        nop.sync_info = SyncInfo(on_wait=[w], on_update=[])
                        out.append(nop)
                    inst.sync_info = SyncInfo(
                        on_wait=[waits[-1]], on_update=list(si.on_update)
                    )
                out.append(inst)
            blk.instructions = out


N_CORES = 8


def _in_maps(queries, keys, values, k_cg, v_cg):
    consts = host_consts()
    nb = queries.shape[0]
    sh = nb // N_CORES
    return [
        {
            "queries": queries[i * sh : (i + 1) * sh],
            "keys": keys[i * sh : (i + 1) * sh],
            "values": values[i * sh : (i + 1) * sh],
            "k_cg": k_cg[i * sh : (i + 1) * sh],
            "v_cg": v_cg[i * sh : (i + 1) * sh],
            **consts,
        }
        for i in range(N_CORES)
    ]


def kernel(queries, keys, values, k_cg, v_cg):
    from concourse.bass_utils import run_bass_kernel_spmd

    queries = np.ascontiguousarray(np.asarray(queries, dtype=np.float32))
    keys = np.ascontiguousarray(np.asarray(keys, dtype=np.float32))
    values = np.ascontiguousarray(np.asarray(values, dtype=np.float32))
    k_cg = np.ascontiguousarray(np.asarray(k_cg, dtype=np.float32))
    v_cg = np.ascontiguousarray(np.asarray(v_cg, dtype=np.float32))

    nb = queries.shape[0]
    in_maps = _in_maps(queries, keys, values, k_cg, v_cg)
    nc = _get_nc()
    res = run_bass_kernel_spmd(nc, in_maps, core_ids=list(range(N_CORES)))
    total = 0.0
    for i in range(N_CORES):
        part = res.results[i]["out"]
        total += float(part[0, 0]) + float(part[0, 1])
    return np.float32(total / nb)


# revision 29
# speedup vs baseline: 1.3765x; 1.0088x over previous
"""Trainium2 Bass kernel for nn_ForceMatchingLoss (batch-data-parallel over 8 NeuronCores).

Full inputs (B=256) are sharded along the batch dimension: core i computes
batches [32*i, 32*i+32) and returns [sum_b fd_b, sum_b cons_b]; the host
sums the 8 partials and divides by 256 (the loss is a batch mean, so the
"all-reduce" is a trivial host-side sum of 8 scalars).

v3: fp8 DMA-cast loads (SWDGE casts fp32->fp8 in flight, no on-chip cast
ops), out/kbar computed in the d-major direction with fp8 DoubleRow
matmuls (M=128 instead of M=16: 4096 -> ~400 PE cycles/group) then
transposed back in two batched PE transposes, coarse-grained chain in a
32-padded partition layout (kills the 4-per-group partition-restage DMAs
and runs the cg jacobian in fp8), and per-batch jacobian reductions fused
into single accum ops reading PSUM directly.
"""

import numpy as np


# ---------------------------------------------------------------------------
# Workaround for this walrus build: CTRL-type instructions (Drain) only accept
# a single sync-wait; TileContext's tail drain aggregates one wait per logical
# processor.  Split the waits across a chain of drains.
# ---------------------------------------------------------------------------
def _install_drain_fix():
    import concourse.tile as tile
    from bass_rust import ScopedClock, SyncInfo

    if getattr(tile.TileContext, "_drain_fix_installed", False):
        return

    def _drain_and_barrier(self, tick_clock, wait_clock):
        drain_inst = self.nc.sync.drain()
        wait_clock.add_sem_waits(
            drain_inst.ins, ScopedClock({None: tick_clock.global_clock})
        )
        si = drain_inst.ins.sync_info
        waits = list(si.on_wait) if si is not None else []
        if len(waits) > 1:
            drain_inst.ins.sync_info = SyncInfo(
                on_wait=waits[:1], on_update=list(si.on_update)
            )
            for i in range(1, len(waits)):
                d = self.nc.sync.drain()
                d.ins.sync_info = SyncInfo(on_wait=waits[i : i + 1], on_update=[])

        self.nc.all_engine_barrier()
        popped = self.nc._tile_sem_poison_stack.pop()
        assert popped is self._sem_poison
        self.nc.clear_and_free_semaphores(list(self.sems.allocated().values()))
        self.nc.all_engine_barrier()

    tile.TileContext._drain_and_barrier = _drain_and_barrier
    tile.TileContext._drain_fix_installed = True


import concourse.bass as bass
import concourse.tile as tile
from concourse import mybir
from concourse.bass import ds, ts
from concourse.masks import make_identity

FP32 = mybir.dt.float32
BF16 = mybir.dt.bfloat16
F32R = mybir.dt.float32r
F8 = mybir.dt.float8e4
DR = mybir.MatmulPerfMode.DoubleRow
AX = mybir.AxisListType
ALU = mybir.AluOpType
ACTF = mybir.ActivationFunctionType

B = 32          # batches per core
Q = 16
S = 512
M = 8
D = 128
NCH = 4         # s chunks of 128
GB = 4          # batches per group (32-row bands in the scores psum tile)
NG = B // GB    # 8 groups
SCALE = float(D) ** -0.5
EPS = 1e-8
QD = float(Q * D)
# Uniform psum->sbuf scale: with LAM = sqrt(64*SCALE) on both the out and
# kbar halves (and the jacobians globally negated via the vc / vccg signs),
# jac2 = (LAM out)^T (LAM kbar) lands with exactly the +64*S weight the
# negated 64*jac1 needs, so each ok tile needs only ONE scaled copy.
LAM = float(np.sqrt(64.0 * SCALE))
PREFETCH = 3    # groups of k/v loads in flight ahead of compute


def r(ap):
    return ap.bitcast(F32R)


def host_consts():
    """Constant mask tensors supplied as extra kernel inputs."""
    # cmask [128, 32]: block-diagonal valid mask for the batched cg scores.
    # row = 32*j + qi (qi valid when < 16), col = 8*j' + m; 1 iff j==j', qi<16.
    cm = np.zeros((128, GB * M), dtype=np.float32)
    for j in range(GB):
        cm[32 * j : 32 * j + Q, M * j : M * j + M] = 1.0
    # qmask1 [128,1]: per-partition valid-row mask (junk rows zeroed).
    q1 = np.zeros((128, 1), dtype=np.float32)
    for j in range(GB):
        q1[32 * j : 32 * j + Q] = 1.0
    return {"cmask": cm, "qmask1": q1}


def build_nc():
    nc = bass.Bass("TRN2", target_bir_lowering=False, debug=False)
    q_d = nc.dram_tensor("queries", [B, Q, D], FP32, kind="ExternalInput").ap()
    k_d = nc.dram_tensor("keys", [B, S, D], FP32, kind="ExternalInput").ap()
    v_d = nc.dram_tensor("values", [B, S, D], FP32, kind="ExternalInput").ap()
    kcg_d = nc.dram_tensor("k_cg", [B, M, D], FP32, kind="ExternalInput").ap()
    vcg_d = nc.dram_tensor("v_cg", [B, M, D], FP32, kind="ExternalInput").ap()
    cm_d = nc.dram_tensor("cmask", [128, GB * M], FP32, kind="ExternalInput").ap()
    q1_d = nc.dram_tensor("qmask1", [128, 1], FP32, kind="ExternalInput").ap()
    out_d = nc.dram_tensor("out", [1, 2], FP32, kind="ExternalOutput").ap()

    with tile.TileContext(nc) as tc:
        with (
            tc.tile_pool(name="const", bufs=1) as constp,
            tc.tile_pool(name="kvb", bufs=1 + PREFETCH) as kvbp,
            tc.tile_pool(name="kt", bufs=5) as ktp,
            tc.tile_pool(name="sm", bufs=3) as smp,
            tc.tile_pool(name="ptp", bufs=3) as ptp,
            tc.tile_pool(name="vcp", bufs=3) as vcp,
            tc.tile_pool(name="small", bufs=4) as smallp,
            tc.tile_pool(name="ok", bufs=3) as okp,
            tc.tile_pool(name="scr", bufs=4) as scrp,
            tc.tile_pool(name="psSC", bufs=2, space="PSUM") as psSC,
            tc.tile_pool(name="psT", bufs=2, space="PSUM") as psT,
            tc.tile_pool(name="psCG", bufs=2, space="PSUM") as psCG,
            tc.tile_pool(name="psJ", bufs=2, space="PSUM") as psJ,
        ):
            # ---------- k/v prefetch (SWDGE DMA-cast fp32 -> fp8) ----------
            kvq = {}

            def load_group(gg):
                kb = kvbp.tile([128, GB, NCH, 128], F8, tag="kb")
                nc.gpsimd.dma_start(
                    out=kb,
                    in_=k_d[ds(GB * gg, GB)].rearrange(
                        "b (p c) d -> p b c d", c=NCH
                    ),
                )
                vb = kvbp.tile([128, GB, NCH, 128], F8, tag="vb")
                nc.gpsimd.dma_start(
                    out=vb,
                    in_=v_d[ds(GB * gg, GB)].rearrange(
                        "b (p c) d -> p b c d", c=NCH
                    ),
                )
                kvq[gg] = (kb, vb)

            for gg in range(min(PREFETCH, NG)):
                load_group(gg)

            # ---------- constants ----------
            ident = constp.tile([128, 128], FP32)
            make_identity(nc, ident)
            identb = constp.tile([128, 128], BF16)
            nc.scalar.copy(identb, ident)
            identf8 = constp.tile([128, 128], F8)
            nc.scalar.copy(identf8, ident)
            ones1 = constp.tile([128, 1], FP32)
            nc.vector.memset(ones1, 1.0)
            # accumulator columns: per group g, [dot 12g:12g+4 | d2 +4 | c2 +8];
            # cons at 96+g
            accum = constp.tile([128, 128], FP32)
            nc.gpsimd.memset(accum, 0.0)
            cmask = constp.tile([128, GB * M], FP32)
            nc.sync.dma_start(out=cmask, in_=cm_d)
            qmask1 = constp.tile([128, 1], FP32)
            nc.sync.dma_start(out=qmask1, in_=q1_d)
            qmaskL2 = constp.tile([128, 1], FP32)
            nc.vector.tensor_scalar_mul(qmaskL2, qmask1, LAM)

            # scps junk bands are never written by the M=16 score matmuls;
            # they must hold finite data before the full-tile exp (cols
            # 0:256 get overwritten by the okT reuse each group, which keeps
            # them finite thereafter).  Clear every rotating buffer once.
            for _ in range(2):
                sc0 = psSC.tile([128, 512], FP32, tag="scps")
                nc.vector.memset(sc0, 0.0)

            # ---------- prologue: queries ----------
            # dense load [128 = (b2 q), 4 t, 128 d] in ONE DMA, then transpose
            # and spread into the junk-band layout on-chip.
            q_sb = constp.tile([128, 4, 128], FP32)
            nc.sync.dma_start(
                out=q_sb,
                in_=q_d.rearrange("(t b2) q d -> (b2 q) t d", t=4),
            )
            # qJ [128 d, 8 g, 128 = 4b x 32 cols] fp8, softmax scale folded in
            qJ = constp.tile([128, NG, 128], F8)
            nc.gpsimd.memset(qJ, 0.0)
            for t in range(4):
                qtps = psSC.tile([128, 512], FP32, tag="scps")
                nc.tensor.transpose(qtps[:, 0:128], q_sb[:, t, :], ident)
                # qtps cols = b2*16+q for batches t*8..t*8+8 = groups 2t, 2t+1
                for gg in range(2):
                    g2 = 2 * t + gg
                    nc.scalar.activation(
                        out=qJ[:, g2, :].rearrange("p (j w) -> p j w", w=32)[
                            :, :, 0:Q
                        ],
                        in_=qtps[:, ds(64 * gg, 64)].rearrange(
                            "p (j w) -> p j w", w=Q
                        ),
                        func=ACTF.Copy,
                        scale=SCALE,
                    )

            # ---------- prologue: coarse-grained tensors ----------
            # kcgT [128 d, 2 t, 128 = (8 b2) x (8 m)] bf16 (no scale)
            kcg_sb = constp.tile([128, 2, 128], FP32)
            nc.sync.dma_start(
                out=kcg_sb,
                in_=kcg_d.rearrange("(t b2) m d -> (b2 m) t d", t=2),
            )
            kcgT = constp.tile([128, 2, 128], F8)
            kcgtps = psSC.tile([128, 512], FP32, tag="scps")
            for t in range(2):
                nc.tensor.transpose(kcgtps[:, ts(t, 128)], kcg_sb[:, t, :], ident)
            nc.scalar.copy(kcgT[:], kcgtps[:, 0:256].rearrange("p (t x) -> p t x", t=2))

            # Padded cg layout: partition row 32*bl + m holds (batch 4g+bl,
            # coarse key m); junk partitions stay zero so the block-diagonal
            # okcg matmul and the per-batch jac1cg slices read clean data.
            cgkvJ32 = constp.tile([128, NG, 2, 128], FP32)
            nc.gpsimd.memset(cgkvJ32, 0.0)
            kcg8 = constp.tile([128, NG, 128], F8)
            for bl in range(GB):
                src_k = kcg_d.rearrange("(g bl) m d -> bl m g d", bl=GB)[ds(bl, 1)]
                src_v = vcg_d.rearrange("(g bl) m d -> bl m g d", bl=GB)[ds(bl, 1)]
                nc.sync.dma_start(
                    out=r(cgkvJ32[ds(32 * bl, M), :, 0, :]), in_=r(src_k)
                )
                nc.sync.dma_start(
                    out=r(cgkvJ32[ds(32 * bl, M), :, 1, :]), in_=r(src_v)
                )
                nc.gpsimd.dma_start(out=kcg8[ds(32 * bl, M), :, :], in_=src_k)

            # persistent padded pcg tile: valid cols 32*bl + m, junk cols
            # zeroed once and never written again.
            pcgn128 = constp.tile([128, 128], FP32)
            nc.gpsimd.memset(pcgn128, 0.0)

            # ---------- main loop over groups of 4 batches ----------
            # Software-pipelined: stage A (kT transposes + scores + cg
            # scores — PE-heavy, depends only on the prefetched kb) runs one
            # group ahead of stage B, so the PE chews on group g+1's
            # transposes while group g's softmax/copy chains run on
            # Act/DVE/Pool.

            def stage_a(g):
                kb, vb = kvq.pop(g)
                kts = []
                for j in range(GB):
                    ktps = psT.tile([128, 1024], F8, tag="ktps")
                    ktv = ktps.rearrange("p (x two) -> p x two", two=2)[:, :, 0]
                    for c in range(NCH):
                        nc.tensor.transpose(
                            ktv[:, ts(c, 128)], kb[:, j, c, :], identf8
                        )
                    kt = ktp.tile([128, NCH, 128], F8, tag="kt")
                    ktv_r = ktv.rearrange("p (c x) -> p c x", c=NCH)
                    if j % 2 == 0:
                        nc.vector.tensor_copy(kt[:], ktv_r)
                    else:
                        nc.scalar.copy(kt[:], ktv_r)
                    kts.append(kt)

                # scores: batch j -> rows [32j, 32j+16)
                scps = psSC.tile([128, 512], FP32, tag="scps")
                for j in range(GB):
                    nc.tensor.matmul(
                        scps[ds(32 * j, Q), :],
                        lhsT=qJ[:, g, ds(32 * j, Q)],
                        rhs=kts[j].rearrange("p c x -> p (c x)"),
                        start=True,
                        stop=True,
                        perf_mode=mybir.MatmulPerfMode.DoublePixel,
                        tile_position=(0, 32 * j),
                        skip_group_check=True,
                    )

                # cg scores into the merged cg psum tile:
                # cols [0:64 cgs | 64:320 okcgps | 320:448 pcgps].
                cgall = psCG.tile([128, 512], FP32, tag="cgall")
                nc.tensor.matmul(
                    cgall[:, 0 : GB * M],
                    lhsT=qJ[:, g, :],
                    rhs=kcgT[:, g // 4, ds(8 * ((GB * g) % 16), GB * M)],
                    start=True,
                    stop=True,
                    skip_group_check=True,
                )
                return kb, vb, scps, cgall

            staged = {0: stage_a(0)}

            for g in range(NG):
                if g + PREFETCH < NG:
                    load_group(g + PREFETCH)
                if g + 1 < NG:
                    staged[g + 1] = stage_a(g + 1)
                kb, vb, scps, cgall = staged.pop(g)
                cgs = cgall[:, 0:64]
                okcgps = cgall[:, 64:320]
                pcgps = cgall[:, 320:448]

                # softmax (no max subtraction: |scores| <= ~7)
                ptil = smp.tile([128, 512], BF16, tag="ptil")
                z = smallp.tile([128, 1], FP32, tag="z")
                nc.scalar.activation(out=ptil, in_=scps, func=ACTF.Exp, accum_out=z)
                zr = smallp.tile([128, 1], FP32, tag="zr")
                nc.vector.reciprocal(zr, z)
                ptn = smp.tile([128, 512], F8, tag="ptn")
                nc.vector.tensor_scalar(
                    out=ptn, in0=ptil, scalar1=zr, scalar2=64.0,
                    op0=ALU.mult, op1=ALU.mult,
                )

                # cg softmax chain (Act/DVE small ops)
                ecg = scrp.tile([128, GB * M], FP32, tag="ecg")
                nc.scalar.activation(out=ecg, in_=cgs[:, 0 : GB * M], func=ACTF.Exp)
                em = scrp.tile([128, GB * M], FP32, tag="em")
                zcg = smallp.tile([128, 1], FP32, tag="zcg")
                nc.vector.scalar_tensor_tensor(
                    out=em, in0=ecg, scalar=1.0, in1=cmask,
                    op0=ALU.mult, op1=ALU.mult, accum_out=zcg,
                )
                nc.vector.tensor_scalar_add(zcg, zcg, 1e-20)
                zcgr = smallp.tile([128, 1], FP32, tag="zcgr")
                nc.vector.reciprocal(zcgr, zcg)
                nc.scalar.activation(
                    out=pcgn128.rearrange("p (bl w) -> p bl w", w=32)[:, :, 0:M],
                    in_=em.rearrange("p (bl m) -> p bl m", m=M),
                    func=ACTF.Copy,
                    scale=zcgr,
                )

                # pT via fp8 PE transpose
                ptps = psT.tile([128, 1024], F8, tag="ktps")
                ptv = ptps.rearrange("p (x two) -> p x two", two=2)[:, :, 0]
                for c in range(NCH):
                    nc.tensor.transpose(
                        ptv[:, ts(c, 128)], ptn[:, ts(c, 128)], identf8
                    )
                pT = ptp.tile([128, NCH, 128], F8, tag="pT")
                nc.vector.tensor_copy(pT[:], ptv.rearrange("p (c x) -> p c x", c=NCH))

                # outT/kbarT: fp8 DR, M=128 (d on partitions), 16 tiny
                # matmuls.  The score psum tile is dead after exp, so its
                # cols [0:128] host the DR outputs and [128:256] (bitcast to
                # bf16) host the transposed-back okKO.
                okTps = scps[:, 0:128].rearrange(
                    "p (h j q) -> p h j q", h=2, j=GB
                )
                for j in range(GB):
                    for h, src in ((0, kb), (1, vb)):
                        for cp in range(2):
                            nc.tensor.matmul(
                                okTps[:, h, j, :],
                                lhsT=src[:, j, ds(2 * cp, 2), :],
                                rhs=pT[:, ds(2 * cp, 2), ds(32 * j, Q)],
                                start=(cp == 0),
                                stop=(cp == 1),
                                perf_mode=DR,
                                skip_group_check=True,
                            )

                # c = sum_q p (valid q rows only); SCALE folded into vc below
                c_t = smallp.tile([128, NCH, GB], FP32, tag="c_t")
                nc.vector.tensor_reduce(
                    out=c_t,
                    in_=pT.rearrange("p c (j w) -> p c j w", j=GB)[:, :, :, 0:Q],
                    axis=AX.X,
                    op=ALU.add,
                )
                nc.gpsimd.tensor_scalar_mul(c_t, c_t, -SCALE)
                vcs = []
                for j in range(GB):
                    vc = vcp.tile([128, NCH, 128], F8, tag="vc")
                    veng = nc.vector if j % 2 == 0 else nc.gpsimd
                    veng.tensor_tensor(
                        out=vc,
                        in0=vb[:, j, :, :],
                        in1=c_t[:, :, ds(j, 1)].broadcast_to([128, NCH, 128]),
                        op=ALU.mult,
                    )
                    vcs.append(vc)

                # transpose outT/kbarT back to the q-banded layout: staging
                # copy (junk cols zero) then one batched transpose per half.
                okTs = okp.tile([128, 2, GB, 32], BF16, tag="okTs")
                nc.gpsimd.memset(okTs, 0.0)
                nc.vector.tensor_copy(okTs[:, :, :, 0:Q], okTps)
                okKO = scps[:, 128:256].bitcast(BF16).rearrange(
                    "p (h x) -> p h x", h=2
                )
                nc.tensor.transpose(okKO[:, 0, :], okTs[:, 0, :, :], identb)
                nc.tensor.transpose(okKO[:, 1, :], okTs[:, 1, :, :], identb)
                # okb [128, 256] bf16 = LAM * [kbar | out] (psum junk rows
                # are zero, so one scalar-scaled copy covers both halves)
                okb = okp.tile([128, 256], BF16, tag="okb")
                nc.scalar.activation(
                    out=okb, in_=okKO.rearrange("p h x -> p (h x)"),
                    func=ACTF.Copy, scale=LAM / 64.0,
                )

                # ---- coarse-grained out/kbar (padded block-diagonal) ----
                nc.tensor.transpose(pcgps, pcgn128, ident)
                pcgTs = scrp.tile([128, 128], FP32, tag="pcgTs")
                nc.scalar.copy(r(pcgTs[:]), pcgps)
                ccg32 = smallp.tile([128, 1], FP32, tag="ccg")
                nc.vector.tensor_reduce(out=ccg32, in_=pcgTs, axis=AX.X, op=ALU.add)
                vccg32 = scrp.tile([128, 128], BF16, tag="vccg")
                nc.vector.tensor_scalar(
                    out=vccg32,
                    in0=cgkvJ32[:, g, 1, :],
                    scalar1=ccg32,
                    scalar2=-64.0 * SCALE,
                    op0=ALU.mult,
                    op1=ALU.mult,
                )
                nc.tensor.matmul(
                    okcgps,
                    lhsT=r(pcgTs),
                    rhs=r(cgkvJ32[:, g, :, :].rearrange("p a x -> p (a x)")),
                    start=True,
                    stop=True,
                    skip_group_check=True,
                )
                okcg = okp.tile([128, 256], BF16, tag="okcg")
                nc.scalar.activation(
                    out=okcg, in_=okcgps, func=ACTF.Copy, scale=qmaskL2,
                )

                # consistency for the whole group (junk rows are zero)
                dif = scrp.tile([128, 128], BF16, tag="dif")
                nc.vector.tensor_sub(dif, okb[:, 128:256], okcg[:, 128:256])
                scc = scrp.tile([128, 128], BF16, tag="scc")
                nc.vector.scalar_tensor_tensor(
                    out=scc, in0=dif, scalar=1.0, in1=dif,
                    op0=ALU.mult, op1=ALU.mult,
                    accum_out=accum[:, ds(96 + g, 1)],
                )

                # ---- per-batch jacobians (2 per psum tile) ----
                jall = scrp.tile([128, GB, 256], BF16, tag="jall")
                for pair in range(2):
                    jp = psJ.tile([128, 2, 256], FP32, tag="jd")
                    for jj in range(2):
                        j = 2 * pair + jj
                        # dense jac (x64): fp8 DR chunk-pairs + out^T(-64 s kbar)
                        for cp in range(2):
                            nc.tensor.matmul(
                                jp[:, jj, 0:128],
                                lhsT=vcs[j][:, ds(2 * cp, 2), :],
                                rhs=kb[:, j, ds(2 * cp, 2), :],
                                start=(cp == 0),
                                stop=False,
                                perf_mode=DR,
                                skip_group_check=True,
                            )
                        nc.tensor.matmul(
                            jp[:, jj, 0:128],
                            lhsT=okb[ds(32 * j, Q), 128:256],
                            rhs=okb[ds(32 * j, Q), 0:128],
                            start=False,
                            stop=True,
                            tile_position=(32 * j, 0),
                            skip_group_check=True,
                        )
                        # cg jac: fp8 jac1 + bf16 jac2, same psum region
                        nc.tensor.matmul(
                            jp[:, jj, 128:256],
                            lhsT=vccg32[ds(32 * j, M), :],
                            rhs=kcg8[ds(32 * j, M), g, :],
                            start=True,
                            stop=False,
                            tile_position=(32 * j, 0),
                            skip_group_check=True,
                        )
                        nc.tensor.matmul(
                            jp[:, jj, 128:256],
                            lhsT=okcg[ds(32 * j, Q), 128:256],
                            rhs=okcg[ds(32 * j, Q), 0:128],
                            start=False,
                            stop=True,
                            tile_position=(32 * j, 0),
                            skip_group_check=True,
                        )
                    # bounce the pair to SBUF bf16 for the reductions
                    nc.scalar.copy(jall[:, ds(2 * pair, 2), :], jp)

                # group-level dot/norm reductions off SBUF
                pr = scrp.tile([128, GB, 128], BF16, tag="pr")
                nc.vector.tensor_tensor(
                    out=pr, in0=jall[:, :, 0:128], in1=jall[:, :, 128:256],
                    op=ALU.mult,
                )
                nc.vector.tensor_reduce(
                    out=accum[:, ds(12 * g, GB)], in_=pr, axis=AX.X, op=ALU.add
                )
                sq = scrp.tile([128, GB, 256], BF16, tag="sq")
                nc.scalar.activation(out=sq, in_=jall, func=ACTF.Square)
                nc.vector.tensor_reduce(
                    out=accum[:, ds(12 * g + 4, 8)],
                    in_=sq.rearrange("p j (h x) -> p (j h) x", h=2),
                    axis=AX.X,
                    op=ALU.add,
                )

            # ---------- final reduction ----------
            rps = psJ.tile([1, 128], FP32, tag="jd")
            nc.tensor.matmul(
                rps, lhsT=ones1, rhs=accum, start=True, stop=True,
                skip_group_check=True,
            )
            row = constp.tile([1, 128], FP32)
            nc.scalar.copy(row, rps)
            rw = row[:, 0:96].rearrange("o (g x) -> o g x", x=12)
            sqv = rw[:, :, 4:12].rearrange("o g (j h) -> o g j h", h=2)
            f1 = constp.tile([1, 32], FP32)
            f1v = f1.rearrange("o (g f) -> o g f", f=GB)
            nc.vector.tensor_tensor(
                out=f1v, in0=sqv[:, :, :, 0], in1=sqv[:, :, :, 1], op=ALU.mult
            )
            nc.scalar.activation(out=f1, in_=f1, func=ACTF.Sqrt)
            nc.vector.tensor_scalar_add(f1, f1, EPS)
            f2 = constp.tile([1, 32], FP32)
            f2v = f2.rearrange("o (g f) -> o g f", f=GB)
            nc.vector.reciprocal(f2, f1)
            nc.vector.tensor_tensor(
                out=f2v, in0=rw[:, :, 0:4], in1=f2v, op=ALU.mult
            )
            csum = constp.tile([1, 1], FP32)
            nc.vector.tensor_reduce(out=csum, in_=f2, axis=AX.X, op=ALU.add)
            msum = constp.tile([1, 1], FP32)
            nc.vector.tensor_reduce(
                out=msum, in_=row[:, 96:128], axis=AX.X, op=ALU.add
            )
            part = constp.tile([1, 2], FP32)
            nc.vector.tensor_scalar(
                out=part[:, 0:1], in0=csum, scalar1=-1.0, scalar2=float(B),
                op0=ALU.mult, op1=ALU.add,
            )
            nc.vector.tensor_scalar_mul(
                part[:, 1:2], msum, 1.0 / (LAM * LAM * QD)
            )
            nc.sync.dma_start(out=out_d, in_=part)

    return nc


_NC_CACHE = {}


def _get_nc():
    if "nc" not in _NC_CACHE:
        _install_drain_fix()
        nc = build_nc()
        _split_waits(nc)
        _NC_CACHE["nc"] = nc
    return _NC_CACHE["nc"]


def _split_waits(nc):
    """This walrus accepts only one sync-wait per instruction; move extras
    onto same-engine NoOps inserted just before."""
    from concourse import mybir
    from bass_rust import SyncInfo

    for f in nc.m.functions:
        for blk in f.blocks:
            insts = list(blk.instructions)
            out = []
            for inst in insts:
                si = inst.sync_info
                waits = list(si.on_wait) if si is not None else []
                if len(waits) > 1:
                    for wi, w in enumerate(waits[:-1]):
                        nop = mybir.InstNoOp(name=f"{inst.name}-wsplit{wi}")
                        nop.engine = inst.engine
                        nop.sync_info = SyncInfo(on_wait=[w], on_update=[])
                        out.append(nop)
                    inst.sync_info = SyncInfo(
                        on_wait=[waits[-1]], on_update=list(si.on_update)
                    )
                out.append(inst)
            blk.instructions = out


N_CORES = 8


def _in_maps(queries, keys, values, k_cg, v_cg):
    consts = host_consts()
    nb = queries.shape[0]
    sh = nb // N_CORES
    return [
        {
            "queries": queries[i * sh : (i + 1) * sh],
            "keys": keys[i * sh : (i + 1) * sh],
            "values": values[i * sh : (i + 1) * sh],
            "k_cg": k_cg[i * sh : (i + 1) * sh],
            "v_cg": v_cg[i * sh : (i + 1) * sh],
            **consts,
        }
        for i in range(N_CORES)
    ]


def kernel(queries, keys, values, k_cg, v_cg):
    from concourse.bass_utils import run_bass_kernel_spmd

    queries = np.ascontiguousarray(np.asarray(queries, dtype=np.float32))
    keys = np.ascontiguousarray(np.asarray(keys, dtype=np.float32))
    values = np.ascontiguousarray(np.asarray(values, dtype=np.float32))
    k_cg = np.ascontiguousarray(np.asarray(k_cg, dtype=np.float32))
    v_cg = np.ascontiguousarray(np.asarray(v_cg, dtype=np.float32))

    nb = queries.shape[0]
    in_maps = _in_maps(queries, keys, values, k_cg, v_cg)
    nc = _get_nc()
    res = run_bass_kernel_spmd(nc, in_maps, core_ids=list(range(N_CORES)))
    total = 0.0
    for i in range(N_CORES):
        part = res.results[i]["out"]
        total += float(part[0, 0]) + float(part[0, 1])
    return np.float32(total / nb)
